# revision 26
# baseline (speedup 1.0000x reference)
"""Trainium2 Bass kernel for nn_ExpandedResolventFMNet.

Mathematical reformulation (validated in fp64/fp16 against the jax reference):

The reference builds kron(A.T, My) [8192x4096], its Gram [4096^2], resolvent
kron masks, and solves a dense 4096x4096 system.  All of that collapses:

  first        = kron(A A^T, G),              G = My^T My
  second       = kron-sum of 64x64 factors; with X = Mx W the full system is
  M(W)         = S~ W G + LMBDA * sum_d Dd*( (Dd*W) G ) = R~    (* = Hadamard)
  S~           = Mx^T (A A^T) Mx
  R~           = G By A^T Mx,   By = Py fy
  Dd           = resolvent-mask difference matrices (64x64)
  output C     = (Mx W)^T

The device runs the transposed system Y = W^T:

  M'(Y) = G Y S~ + sum_d DdT * (G (DdT * Y)),   C = Y Mx^T

solved by PCG with the exact-kron preconditioner P^-1 x = Gi x Si, where
Gi, Si come from on-device Newton-Schulz iteration (two-hop steps:
X' = X (2I - S X)).  G's symmetry makes every matmul transpose-free, and
Y^T is accumulated in PSUM via matmul against an alpha-scaled identity so
the output needs no transpose.

Fully unsharded: every core redundantly computes the whole answer, so there
are no collectives (SPMD launch skew made the barrier + two AllReduce cost
~64us on the measured core).  All matmuls run in fp16 (single-pass, 1
cycle/row vs fp32's split LOW_HIGH 2-pass) with fp32 PSUM accumulation; CG
state stays fp32 except the search direction p (fp16).  Inputs are cast to
fp16 on the host (halves HBM traffic).  The four big tensors stream through
the gpsimd SWDGE queue, which stripes descriptors over all 16 SDMA engines
(the HWDGE rings only get 5); queue FIFO order gives the x-side strict
priority.  Each partition's data is contiguous in DRAM (125 descriptors
per tensor).
"""

import numpy as np

import concourse.bacc as bacc
import concourse.mybir as mybir
from concourse.bass_utils import run_bass_kernel_spmd
from concourse.tile import TileContext

F32 = mybir.dt.float32
F16 = mybir.dt.float16
F32R = mybir.dt.float32r
K = 64          # spectral basis size
C = 128         # feature channels
V = 5000        # vertices
P = 128         # DMA partition rows
NB = 39         # full contraction chunks (V = P * NB + TAIL)
TAIL = 8        # leftover rows handled by one small matmul
N_CORES = 8
N_ITERS = 6
NEWTON_STEPS_S = 7
NEWTON_STEPS_G = 4
SQRT_LMBDA = 10.0

SHARD = False   # kept for test.py compat; only the unsharded path exists

_PROGRAM_CACHE = {}


def build_program(shard: bool):
    nc = bacc.Bacc("TRN2", num_devices=N_CORES)

    CK = C + K      # concatenated row: [fx | pxT]
    xc_d = nc.dram_tensor("xc", [V, CK], F16, kind="ExternalInput")
    yc_d = nc.dram_tensor("yc", [V, CK], F16, kind="ExternalInput")
    # mx|my|mxT|I [64, 4*64] fp32 (host-concatenated; I avoids the gpsimd
    # affine_select library swap that delayed partition_all_reduce ~13us)
    small_d = nc.dram_tensor("small", [K, 4 * K], F32, kind="ExternalInput")
    ev_d = nc.dram_tensor("ev", [1, 2 * K], F32, kind="ExternalInput")
    out_d = nc.dram_tensor("out", [K, K], F32, kind="ExternalOutput")

    xc_ap = xc_d[0:P * NB, :].rearrange("(p n) c -> p n c", p=P)
    yc_ap = yc_d[0:P * NB, :].rearrange("(p n) c -> p n c", p=P)

    with TileContext(nc) as tc:
        with (
            tc.tile_pool(name="big", bufs=1) as bp,
            tc.tile_pool(name="persist", bufs=1) as sp,
            tc.tile_pool(name="work", bufs=2) as wp,
            tc.tile_pool(name="psum", bufs=2, space="PSUM") as pp,
            tc.tile_pool(name="yacc", bufs=1, space="PSUM") as yp,
            tc.tile_pool(name="scal", bufs=2, space="PSUM") as scp,
        ):
            _sc_state = {"i": 0}

            def sc_tile(shape):
                i = _sc_state["i"]
                _sc_state["i"] += 1
                return scp.tile(shape, F32, tag="sc", name=f"sct{i}")
            _ps_state = {"i": 0}

            def ps_tile(shape):
                i = _ps_state["i"]
                _ps_state["i"] += 1
                return pp.tile(shape, F32, tag=f"ps{i % 2}", name=f"pst{i}")

            # ---------------- input DMA ------------------------------------
            # smalls ride the (otherwise idle) HWDGE queues; the four big
            # tensors stream through gpsimd SWDGE in x-first FIFO order.
            small_t = sp.tile([K, 4 * K], F32)
            ev_t = sp.tile([1, 2 * K], F32)
            xc_t = bp.tile([P, NB, CK], F16)
            yc_t = bp.tile([P, NB, CK], F16)
            xtl = sp.tile([TAIL, CK], F16)
            ytl = sp.tile([TAIL, CK], F16)
            H = NB // 2
            nc.sync.dma_start(small_t, small_d[:, :])
            nc.sync.dma_start(ev_t, ev_d[:, :])
            nc.sync.dma_start(xtl, xc_d[P * NB:V, :])
            nc.sync.dma_start(ytl, yc_d[P * NB:V, :])
            nc.gpsimd.dma_start(xc_t[:, 0:H, :], xc_ap[:, 0:H, :])
            nc.gpsimd.dma_start(xc_t[:, H:NB, :], xc_ap[:, H:NB, :])
            nc.gpsimd.dma_start(yc_t[:, 0:H, :], yc_ap[:, 0:H, :])
            nc.gpsimd.dma_start(yc_t[:, H:NB, :], yc_ap[:, H:NB, :])

            mx_s = small_t[:, 0:K]
            my_s = small_t[:, K:2 * K]
            mxT_s = small_t[:, 2 * K:3 * K]
            id64 = small_t[:, 3 * K:4 * K]

            # fp16 copies of the small matrices (scalar engine)
            m16 = sp.tile([K, 3 * K], F16)
            nc.scalar.copy(m16[:, 0:K], mx_s)
            nc.scalar.copy(m16[:, K:2 * K], my_s)
            nc.scalar.copy(m16[:, 2 * K:3 * K], mxT_s)
            mx16 = m16[:, 0:K]
            my16 = m16[:, K:2 * K]
            mxT16 = m16[:, 2 * K:3 * K]

            id16 = sp.tile([K, K], F16)
            nc.scalar.copy(id16, id64)
            ones_row = sp.tile([1, K], F32)
            nc.vector.memset(ones_row, 1.0)
            ones_row16 = sp.tile([1, K], F16)
            nc.scalar.copy(ones_row16, ones_row)
            ones_col = sp.tile([K, 1], F32)
            nc.vector.memset(ones_col, 1.0)
            ones_col16 = sp.tile([K, 1], F16)
            nc.scalar.copy(ones_col16, ones_col)

            def preduce(acc16, tag):
                """sum over partitions: fp16 [K,1] -> [1,1] PSUM via PE
                matmul (avoids the gpsimd custom-op library + ~12us load).
                Feeder values are pre-scaled by 2^-8 to fit fp16; the scale
                cancels in every alpha/beta ratio."""
                t_p = sc_tile([1, 1])
                nc.tensor.matmul(t_p, acc16, ones_col16[:, :])
                return t_p

            def bcast(s11_16, tag):
                """fp16 [1,1] SBUF -> [K,1] PSUM broadcast via PE matmul"""
                b_p = sc_tile([K, 1])
                nc.tensor.matmul(b_p, ones_row16[:, :], s11_16)
                return b_p

            # ---------------- G = My^T My (early; data lands first) --------
            g_p = ps_tile([K, K])
            nc.tensor.matmul(g_p, my16, my16)
            g16 = sp.tile([K, K], F16, tag="g16", name="g16")
            nc.vector.tensor_copy(g16, g_p)

            # resolvent scalars: ev = [ex | ey]; t = ev/max(ev); im = 1/(1+t)
            # re = sqrt(t)*im; both scaled by sqrt(LMBDA)
            evmax = sp.tile([1, 1], F32)
            nc.vector.tensor_reduce(evmax, ev_t, mybir.AxisListType.X,
                                    mybir.AluOpType.max)
            evrec = sp.tile([1, 1], F32)
            nc.vector.reciprocal(evrec, evmax)
            t_t = sp.tile([1, 2 * K], F32)
            nc.vector.tensor_scalar_mul(t_t, ev_t, evrec)
            tp1 = sp.tile([1, 2 * K], F32)
            nc.vector.tensor_scalar_add(tp1, t_t, 1.0)
            im_t = sp.tile([1, 2 * K], F32)
            nc.vector.reciprocal(im_t, tp1)
            sq_t = sp.tile([1, 2 * K], F32)
            nc.scalar.sqrt(sq_t, t_t)
            re_t = sp.tile([1, 2 * K], F32)
            nc.vector.tensor_mul(re_t, sq_t, im_t)
            nc.vector.tensor_scalar_mul(re_t, re_t, SQRT_LMBDA)
            nc.vector.tensor_scalar_mul(im_t, im_t, SQRT_LMBDA)

            # Newton-Schulz inverse (S symmetric PD), two-hop steps:
            #   B = 2I - S X  (DVE STT, fp16 out);  X' = X B  (X symmetric).
            # interleave(j) fills the PE-queue gaps with projection matmuls.
            def newton_inverse(mat_p, s16, tag, steps, interleave=None):
                # X0 = I / ||S||_F  (PE partition-reduce; no gpsimd library)
                prod = wp.tile([K, K], F32, tag="dot_dm", name=f"{tag}_sq")
                sqa = sp.tile([K, 1], F16, tag=f"{tag}_rs", name=f"{tag}_rs")
                with nc.allow_low_precision(reason="fp16 PE-reduce feeder"):
                    nc.vector.scalar_tensor_tensor(
                        prod, s16, 2.0 ** -8, s16,
                        op0=mybir.AluOpType.mult, op1=mybir.AluOpType.mult,
                        accum_out=sqa)
                tf_p = preduce(sqa, tag)       # ||S||_F^2 * 2^-8
                sf = sp.tile([1, 1], F32, tag=f"{tag}_sf", name=f"{tag}_sf")
                nc.scalar.sqrt(sf, tf_p)       # ||S||_F * 2^-4
                rec = sp.tile([1, 1], F32, tag=f"{tag}_rc", name=f"{tag}_rc")
                nc.vector.reciprocal(rec, sf)
                alr = sp.tile([1, 1], F16, tag=f"{tag}_al", name=f"{tag}_al")
                nc.vector.tensor_scalar_mul(alr, rec, 2.0 ** -4)  # 1/||S||_F
                al_bc = bcast(alr, tag)
                x_s = sp.tile([K, K], F16, tag=f"{tag}_x0", name=f"{tag}_x0")
                nc.vector.tensor_scalar_mul(x_s, id64, al_bc)
                it_i = 0
                for it in range(steps):
                    t1 = ps_tile([K, K])
                    nc.tensor.matmul(t1, s16, x_s)            # S X (S sym)
                    if interleave is not None:
                        interleave(it_i); it_i += 1
                    b16 = wp.tile([K, K], F16, tag=f"{tag}_b16",
                                  name=f"{tag}_b16")
                    nc.vector.scalar_tensor_tensor(
                        b16, id64, 2.0, t1,
                        op0=mybir.AluOpType.mult,
                        op1=mybir.AluOpType.subtract)         # 2I - S X
                    xn = ps_tile([K, K])
                    nc.tensor.matmul(xn, x_s, b16)            # X (2I - S X)
                    if interleave is not None:
                        interleave(it_i); it_i += 1
                    x_new = sp.tile([K, K], F16, tag=f"{tag}_x{it + 1}",
                                    name=f"{tag}_x{it + 1}")
                    if it == steps - 1:
                        nc.scalar.copy(x_new, xn)
                    else:
                        nc.vector.tensor_copy(x_new, xn)
                    x_s = x_new
                if interleave is not None:
                    interleave(1000)   # flush any remainder
                return x_s

            # ---- x projections interleaved into Newton-G's PE gaps --------
            with tc.tile_pool(name="pacc", bufs=1, space="PSUM") as pacc:
                pj_p = pacc.tile([C, 2 * K], F32)
                at_p = pj_p[:, 0:K]              # A^T  [C,K]
                byt_p = pj_p[:, K:2 * K]         # By^T [C,K]

                xprog = {"n": 0}

                def emit_xproj(upto):
                    while xprog["n"] < min(upto, NB + 1):
                        n = xprog["n"]
                        if n < NB:
                            nc.tensor.matmul(
                                at_p, xc_t[:, n, 0:C], xc_t[:, n, C:CK],
                                start=(n == 0), stop=False)
                        else:
                            nc.tensor.matmul(
                                at_p, xtl[:, 0:C], xtl[:, C:CK],
                                start=False, stop=True)
                        xprog["n"] += 1

                gi16 = newton_inverse(
                    g_p, g16, "gi", NEWTON_STEPS_G,
                    interleave=lambda j: emit_xproj(j * 6))
                emit_xproj(NB + 1)

                at16 = sp.tile([C, K], F16, tag="at16", name="at16")
                nc.vector.tensor_copy(at16, at_p)

                # S~ = Mx^T (A A^T) Mx
                sa_p = ps_tile([K, K])
                nc.tensor.matmul(sa_p, at16, at16)          # A A^T
                sa16 = wp.tile([K, K], F16, tag="sa16", name="sa16")
                nc.vector.tensor_copy(sa16, sa_p)
                h1_p = ps_tile([K, K])
                nc.tensor.matmul(h1_p, sa16, mx16)          # S_A Mx (sym)
                h16 = wp.tile([K, K], F16, tag="h16", name="h16")
                nc.vector.tensor_copy(h16, h1_p)
                st_p = ps_tile([K, K])
                nc.tensor.matmul(st_p, mx16, h16)           # Mx^T S_A Mx
                st16 = sp.tile([K, K], F16, tag="st16", name="st16")
                nc.scalar.copy(st16, st_p)

                # ---- y projections interleaved into Newton-S's gaps -------
                yprog = {"n": 0}

                def emit_yproj(upto):
                    while yprog["n"] < min(upto, NB + 1):
                        n = yprog["n"]
                        if n < NB:
                            nc.tensor.matmul(
                                byt_p, yc_t[:, n, 0:C], yc_t[:, n, C:CK],
                                start=(n == 0), stop=False)
                        else:
                            nc.tensor.matmul(
                                byt_p, ytl[:, 0:C], ytl[:, C:CK],
                                start=False, stop=True)
                        yprog["n"] += 1

                si16 = newton_inverse(
                    st_p, st16, "si", NEWTON_STEPS_S,
                    interleave=lambda j: emit_yproj((j - 3) * 6))
                emit_yproj(NB + 1)

                byt16 = sp.tile([C, K], F16, tag="byt16", name="byt16")
                nc.vector.tensor_copy(byt16, byt_p)

            # D1T[a,i] = re2[a] - re1[i]; D2T likewise from im (emitted late
            # so the tiny mask matmuls never stall the PE queue)
            d12t_s = sp.tile([K, 2 * K], F32)
            for idx, src in enumerate((re_t, im_t)):
                pa = ps_tile([K, K])
                nc.tensor.matmul(pa, src[0:1, K:2 * K], ones_row)  # v2[p]
                pb = ps_tile([K, K])
                nc.tensor.matmul(pb, ones_row, src[0:1, 0:K])      # v1[f]
                ta = wp.tile([K, K], F32, tag=f"dta{idx}", name=f"dta{idx}")
                nc.vector.tensor_copy(ta, pa)
                nc.vector.tensor_sub(d12t_s[:, idx * K:(idx + 1) * K], ta, pb)
            d12v = d12t_s[:, :].rearrange("p (a b) -> p a b", a=2)

            # ---- RHS' = G By A^T Mx (3 matmuls, G-symmetry trick) ---------
            byat_p = ps_tile([K, K])
            nc.tensor.matmul(byat_p, byt16, at16)       # By A^T
            byat16 = wp.tile([K, K], F16, tag="byat16", name="byat16")
            nc.scalar.copy(byat16, byat_p)
            s2_p = ps_tile([K, K])
            nc.tensor.matmul(s2_p, byat16, g16)         # (G ByA^T)^T (G sym)
            s2c = wp.tile([K, K], F16, tag="s2c", name="s2c")
            nc.scalar.copy(s2c, s2_p)
            rp_p = ps_tile([K, K])
            nc.tensor.matmul(rp_p, s2c, mx16)           # G ByA^T Mx

            # ------- PCG state ---------------------------------------------
            # rz_s = [r | z] fp32 (fused alpha-update); p16 fp16 direction;
            # q_s, s_s fp32; y accumulates Y^T in PSUM via matmul.
            rz_s = sp.tile([K, 2 * K], F32)
            w_s = sp.tile([K, K], F32)
            u16 = sp.tile([K, 2 * K], F16)
            p16 = sp.tile([K, K], F16)
            y_p = yp.tile([K, K], F32)
            r_sl = rz_s[:, 0:K]
            z_sl = rz_s[:, K:2 * K]
            qs_v = sp.tile([K, 2 * K], F32)   # [q | s] fused tile
            q_s = qs_v[:, 0:K]
            s_s = qs_v[:, K:2 * K]
            u16v = u16[:, :].rearrange("p (a b) -> p a b", a=2)
            z_bc = z_sl.rearrange("p (o b) -> p o b", o=1).broadcast_to(
                [K, 2, K])

            nc.vector.tensor_copy(r_sl, rp_p)
            r16 = wp.tile([K, K], F16, tag="x16", name="r16i")
            nc.scalar.copy(r16, rp_p)

            def precond_psum(x16, tag):
                """P^-1 x in PSUM via (Gi x)^T = mm(lhsT=x16, rhs=Gi)."""
                ut_p = ps_tile([K, K])
                nc.tensor.matmul(ut_p, x16, gi16)
                ut16 = wp.tile([K, K], F16, tag=f"{tag}_ut", name=f"{tag}_ut")
                nc.scalar.copy(ut16, ut_p)
                v_p = ps_tile([K, K])
                nc.tensor.matmul(v_p, ut16, si16)
                return v_p

            def matvec_z(z16, tag):
                """w = M z -> w_s; w16 returned for the precond."""
                nc.gpsimd.tensor_mul(u16v, d12v, z_bc)   # [D1T*z | D2T*z]
                gzt_p = ps_tile([K, K])
                nc.tensor.matmul(gzt_p, z16, g16)         # (G z)^T
                gzt16 = wp.tile([K, K], F16, tag="mv_gzt", name="mv_gzt")
                nc.scalar.copy(gzt16, gzt_p)
                gu_p = ps_tile([K, 3 * K])
                nc.tensor.matmul(gu_p[:, 0:2 * K], g16, u16)   # G u (both)
                nc.tensor.matmul(gu_p[:, 2 * K:3 * K], gzt16, st16)  # (Gz)S~
                mm_s = wp.tile([K, 2 * K], F32, tag="mv_mm", name="mv_mm")
                nc.vector.tensor_mul(mm_s, d12t_s, gu_p[:, 0:2 * K])  # mask
                a1_s = wp.tile([K, K], F32, tag="mv_a1", name="mv_a1")
                nc.vector.tensor_add(a1_s, mm_s[:, 0:K], mm_s[:, K:2 * K])
                nc.vector.tensor_add(w_s, a1_s, gu_p[:, 2 * K:3 * K])
                w16 = wp.tile([K, K], F16, tag="x16", name=f"{tag}_w16")
                nc.scalar.copy(w16, w_s)
                return w16

            def dot_b(a_ap, b_ap, tag):
                """<a,b> as [1,1] PSUM (DVE accumulate + PE reduce)."""
                prod = wp.tile([K, K], F32, tag="dot_dm", name="dot_dm")
                acc = wp.tile([K, 1], F16, tag=f"{tag}_acc",
                              name=f"{tag}_acc")
                with nc.allow_low_precision(reason="fp16 PE-reduce feeder"):
                    nc.vector.scalar_tensor_tensor(
                        prod, a_ap, 2.0 ** -8, b_ap,
                        op0=mybir.AluOpType.mult, op1=mybir.AluOpType.mult,
                        accum_out=acc)
                return preduce(acc, tag)

            # init: z = P^-1 r; p = z; w = Mz; v = P^-1 w; q = w, s = v
            z0_p = precond_psum(r16, "pcz")
            nc.vector.tensor_copy(z_sl, z0_p)
            nc.scalar.copy(p16, z0_p)
            z16 = wp.tile([K, K], F16, tag="z16", name="z16i")
            nc.scalar.copy(z16, z0_p)
            rz_p = dot_b(r_sl, z_sl, "rz")
            rz11 = wp.tile([1, 1], F32, tag="rz11", name="rz11")
            nc.vector.tensor_copy(rz11, rz_p)
            rzrec = wp.tile([1, 1], F32, tag="rzrec", name="rzrec")
            nc.vector.reciprocal(rzrec, rz_p)
            rzneg = wp.tile([1, 1], F32, tag="rzneg", name="rzneg")
            nc.vector.tensor_scalar_mul(rzneg, rz_p, -1.0)
            w16 = matvec_z(z16, "init")
            nc.vector.tensor_copy(q_s, w_s)
            v_p = precond_psum(w16, "pcv")
            nc.vector.tensor_copy(s_s, v_p)

            for it in range(N_ITERS):
                # ---- alpha = rz/<p,q>; fused [r|z] -= alpha [q|s] ----
                pq_p = dot_b(p16, q_s, "pq")
                pqr = wp.tile([1, 1], F32, tag="pqr", name="pqr")
                nc.vector.reciprocal(pqr, pq_p)
                def emit_y_update():
                    # y^T += alpha p^T (PSUM accumulate, off critical path;
                    # must be emitted BEFORE the p16 beta-update)
                    al = wp.tile([1, 1], F16, tag="al", name="al")
                    nc.vector.tensor_mul(al, rz11, pqr)
                    al_bc = bcast(al, "al")
                    al_s = wp.tile([K, 1], F32, tag="al_s", name="al_s")
                    nc.scalar.copy(al_s, al_bc)
                    ida = wp.tile([K, K], F16, tag="ida", name="ida")
                    nc.scalar.mul(ida, id16, al_s)
                    nc.tensor.matmul(y_p, p16, ida,
                                     start=(it == 0),
                                     stop=(it == N_ITERS - 1))

                if it < N_ITERS - 1:
                    an = wp.tile([1, 1], F16, tag="an", name="an")
                    nc.vector.tensor_mul(an, rzneg, pqr)
                    an_bc = bcast(an, "an")
                    nc.vector.scalar_tensor_tensor(
                        rz_s, qs_v, an_bc, rz_s,
                        op0=mybir.AluOpType.mult, op1=mybir.AluOpType.add)
                    z16 = wp.tile([K, K], F16, tag="z16", name=f"z16_{it}")
                    nc.scalar.copy(z16, z_sl)

                    # ---- rz_new; matvec + precond for next q,s ----
                    rznew_p = dot_b(r_sl, z_sl, "rz")
                    bt = wp.tile([1, 1], F16, tag="bt", name="bt")
                    nc.vector.tensor_mul(bt, rznew_p, rzrec)
                    bt_bc = bcast(bt, "bt")
                    w16 = matvec_z(z16, f"i{it}")
                    emit_y_update()   # uses this iteration's rz11 (alpha)
                    rz11 = wp.tile([1, 1], F32, tag="rz11", name="rz11")
                    nc.vector.tensor_copy(rz11, rznew_p)
                    rzrec = wp.tile([1, 1], F32, tag="rzrec", name="rzrec")
                    nc.vector.reciprocal(rzrec, rznew_p)
                    rzneg = wp.tile([1, 1], F32, tag="rzneg", name="rzneg")
                    nc.vector.tensor_scalar_mul(rzneg, rznew_p, -1.0)
                    if it < N_ITERS - 2:
                        v_s = wp.tile([K, K], F32, tag="v_s", name="v_s")
                        v_p = precond_psum(w16, "pcv")
                        nc.scalar.copy(v_s, v_p)
                    nc.vector.scalar_tensor_tensor(
                        p16, p16, bt_bc, z_sl,
                        op0=mybir.AluOpType.mult, op1=mybir.AluOpType.add)
                    nc.vector.scalar_tensor_tensor(
                        q_s, q_s, bt_bc, w_s,
                        op0=mybir.AluOpType.mult, op1=mybir.AluOpType.add)
                    if it < N_ITERS - 2:
                        nc.vector.scalar_tensor_tensor(
                            s_s, s_s, bt_bc, v_s,
                            op0=mybir.AluOpType.mult, op1=mybir.AluOpType.add)
                else:
                    emit_y_update()
                    break

            # -------- output: C = Y Mx^T  (y_p holds Y^T) ------------------
            y16 = wp.tile([K, K], F16, tag="y16", name="y16")
            nc.scalar.copy(y16, y_p)
            c_p = ps_tile([K, K])
            nc.tensor.matmul(c_p, y16, mxT16)           # (Y^T)^T Mx^T
            c_s = wp.tile([K, K], F32, tag="c_s", name="c_s")
            nc.vector.tensor_copy(c_s, c_p)
            nc.sync.dma_start(out_d[:, :], c_s)

    nc.finalize()
    return nc


def get_program(shard: bool = False):
    if shard not in _PROGRAM_CACHE:
        _PROGRAM_CACHE[shard] = build_program(shard)
    return _PROGRAM_CACHE[shard]


def make_in_maps(inputs, shard: bool = False):
    fx = np.asarray(inputs["feat_x"], np.float32)[0]
    fy = np.asarray(inputs["feat_y"], np.float32)[0]
    pxT = np.asarray(inputs["evecs_trans_x"], np.float32)[0].T
    pyT = np.asarray(inputs["evecs_trans_y"], np.float32)[0].T
    xc = np.ascontiguousarray(
        np.concatenate([fx, pxT], axis=1)).astype(np.float16)
    yc = np.ascontiguousarray(
        np.concatenate([fy, pyT], axis=1)).astype(np.float16)
    mx = np.asarray(inputs["sqrtMk_x"], np.float32)[0]
    my = np.asarray(inputs["sqrtMk_y"], np.float32)[0]
    small = np.ascontiguousarray(np.concatenate(
        [mx, my, mx.T, np.eye(64, dtype=np.float32)], axis=1))
    ev = np.ascontiguousarray(np.concatenate([
        np.asarray(inputs["evals_x"], np.float32)[0],
        np.asarray(inputs["evals_y"], np.float32)[0],
    ])[None, :])
    m = {"xc": xc, "yc": yc, "small": small, "ev": ev}
    return [dict(m) for _ in range(N_CORES)]


def kernel(**inputs) -> np.ndarray:
    nc = get_program(SHARD)
    in_maps = make_in_maps(inputs, SHARD)
    res = run_bass_kernel_spmd(nc, in_maps, core_ids=list(range(N_CORES)))
    out = np.asarray(res.results[0]["out"], dtype=np.float32)
    return out[None]


# revision 27
# speedup vs baseline: 1.0593x; 1.0593x over previous
"""Trainium2 Bass kernel for nn_ExpandedResolventFMNet.

Mathematical reformulation (validated in fp64/fp16 against the jax reference):

The reference builds kron(A.T, My) [8192x4096], its Gram [4096^2], resolvent
kron masks, and solves a dense 4096x4096 system.  All of that collapses:

  first        = kron(A A^T, G),              G = My^T My
  second       = kron-sum of 64x64 factors; with X = Mx W the full system is
  M(W)         = S~ W G + LMBDA * sum_d Dd*( (Dd*W) G ) = R~    (* = Hadamard)
  S~           = Mx^T (A A^T) Mx
  R~           = G By A^T Mx,   By = Py fy
  Dd           = resolvent-mask difference matrices (64x64)
  output C     = (Mx W)^T

The device runs the transposed system Y = W^T:

  M'(Y) = G Y S~ + sum_d DdT * (G (DdT * Y)),   C = Y Mx^T

solved by PCG with the exact-kron preconditioner P^-1 x = Gi x Si, where
Gi, Si come from on-device Newton-Schulz iteration (two-hop steps:
X' = X (2I - S X)).  G's symmetry makes every matmul transpose-free, and
Y^T is accumulated in PSUM via matmul against an alpha-scaled identity so
the output needs no transpose.

Fully unsharded: every core redundantly computes the whole answer, so there
are no collectives (SPMD launch skew made the barrier + two AllReduce cost
~64us on the measured core).  All matmuls run in fp16 (single-pass, 1
cycle/row vs fp32's split LOW_HIGH 2-pass) with fp32 PSUM accumulation; CG
state stays fp32 except the search direction p (fp16).  Inputs are cast to
fp16 on the host (halves HBM traffic).  The four big tensors stream through
the gpsimd SWDGE queue, which stripes descriptors over all 16 SDMA engines
(the HWDGE rings only get 5); queue FIFO order gives the x-side strict
priority.  Each partition's data is contiguous in DRAM (125 descriptors
per tensor).
"""

import numpy as np

import concourse.bacc as bacc
import concourse.mybir as mybir
from concourse.bass_utils import run_bass_kernel_spmd
from concourse.tile import TileContext

F32 = mybir.dt.float32
F16 = mybir.dt.float16
F32R = mybir.dt.float32r
K = 64          # spectral basis size
C = 128         # feature channels
V = 5000        # vertices
P = 128         # DMA partition rows
NB = 39         # full contraction chunks (V = P * NB + TAIL)
TAIL = 8        # leftover rows handled by one small matmul
N_CORES = 8
N_ITERS = 6
NEWTON_STEPS_S = 7
NEWTON_STEPS_G = 4
SQRT_LMBDA = 10.0

SHARD = False   # kept for test.py compat; only the unsharded path exists

_PROGRAM_CACHE = {}


def build_program(shard: bool):
    nc = bacc.Bacc("TRN2", num_devices=N_CORES)

    CK = C + K      # concatenated row: [fx | pxT]
    xc_d = nc.dram_tensor("xc", [V, CK], F16, kind="ExternalInput")
    yc_d = nc.dram_tensor("yc", [V, CK], F16, kind="ExternalInput")
    # mx|my|mxT|I [64, 4*64] fp32 (host-concatenated; I avoids the gpsimd
    # affine_select library swap that delayed partition_all_reduce ~13us)
    small_d = nc.dram_tensor("small", [K, 4 * K], F32, kind="ExternalInput")
    ev_d = nc.dram_tensor("ev", [1, 2 * K], F32, kind="ExternalInput")
    out_d = nc.dram_tensor("out", [K, K], F32, kind="ExternalOutput")

    xc_ap = xc_d[0:P * NB, :].rearrange("(p n) c -> p n c", p=P)
    yc_ap = yc_d[0:P * NB, :].rearrange("(p n) c -> p n c", p=P)

    with TileContext(nc) as tc:
        with (
            tc.tile_pool(name="big", bufs=1) as bp,
            tc.tile_pool(name="persist", bufs=1) as sp,
            tc.tile_pool(name="work", bufs=2) as wp,
            tc.tile_pool(name="psum", bufs=2, space="PSUM") as pp,
            tc.tile_pool(name="yacc", bufs=1, space="PSUM") as yp,
            tc.tile_pool(name="scal", bufs=2, space="PSUM") as scp,
        ):
            _sc_state = {"i": 0}

            def sc_tile(shape):
                i = _sc_state["i"]
                _sc_state["i"] += 1
                return scp.tile(shape, F32, tag="sc", name=f"sct{i}")
            _ps_state = {"i": 0}

            def ps_tile(shape):
                i = _ps_state["i"]
                _ps_state["i"] += 1
                return pp.tile(shape, F32, tag=f"ps{i % 2}", name=f"pst{i}")

            # ---------------- input DMA ------------------------------------
            # smalls ride the (otherwise idle) HWDGE queues; the four big
            # tensors stream through gpsimd SWDGE in x-first FIFO order.
            small_t = sp.tile([K, 4 * K], F32)
            ev_t = sp.tile([1, 2 * K], F32)
            xc_t = bp.tile([P, NB, CK], F16)
            yc_t = bp.tile([P, NB, CK], F16)
            xtl = sp.tile([TAIL, CK], F16)
            ytl = sp.tile([TAIL, CK], F16)
            H = NB // 2
            nc.sync.dma_start(small_t, small_d[:, :])
            nc.sync.dma_start(ev_t, ev_d[:, :])
            nc.sync.dma_start(xtl, xc_d[P * NB:V, :])
            nc.sync.dma_start(ytl, yc_d[P * NB:V, :])
            nc.gpsimd.dma_start(xc_t[:, 0:H, :], xc_ap[:, 0:H, :])
            nc.gpsimd.dma_start(xc_t[:, H:NB, :], xc_ap[:, H:NB, :])
            nc.gpsimd.dma_start(yc_t[:, 0:H, :], yc_ap[:, 0:H, :])
            nc.gpsimd.dma_start(yc_t[:, H:NB, :], yc_ap[:, H:NB, :])

            mx_s = small_t[:, 0:K]
            my_s = small_t[:, K:2 * K]
            mxT_s = small_t[:, 2 * K:3 * K]
            id64 = small_t[:, 3 * K:4 * K]

            # fp16 copies of the small matrices (scalar engine)
            m16 = sp.tile([K, 3 * K], F16)
            nc.scalar.copy(m16[:, 0:K], mx_s)
            nc.scalar.copy(m16[:, K:2 * K], my_s)
            nc.scalar.copy(m16[:, 2 * K:3 * K], mxT_s)
            mx16 = m16[:, 0:K]
            my16 = m16[:, K:2 * K]
            mxT16 = m16[:, 2 * K:3 * K]

            id16 = sp.tile([K, K], F16)
            nc.scalar.copy(id16, id64)
            ones_row = sp.tile([1, K], F32)
            nc.vector.memset(ones_row, 1.0)
            ones_row16 = sp.tile([1, K], F16)
            nc.scalar.copy(ones_row16, ones_row)
            ones_col = sp.tile([K, 1], F32)
            nc.vector.memset(ones_col, 1.0)
            ones_col16 = sp.tile([K, 1], F16)
            nc.scalar.copy(ones_col16, ones_col)

            def preduce(acc16, tag):
                """sum over partitions: fp16 [K,1] -> [1,1] PSUM via PE
                matmul (avoids the gpsimd custom-op library + ~12us load).
                Feeder values are pre-scaled by 2^-8 to fit fp16; the scale
                cancels in every alpha/beta ratio."""
                t_p = sc_tile([1, 1])
                nc.tensor.matmul(t_p, acc16, ones_col16[:, :])
                return t_p

            def bcast(s11_16, tag):
                """fp16 [1,1] SBUF -> [K,1] PSUM broadcast via PE matmul"""
                b_p = sc_tile([K, 1])
                nc.tensor.matmul(b_p, ones_row16[:, :], s11_16)
                return b_p

            # ---------------- G = My^T My (early; data lands first) --------
            g_p = ps_tile([K, K])
            nc.tensor.matmul(g_p, my16, my16)
            g16 = sp.tile([K, K], F16, tag="g16", name="g16")
            nc.vector.tensor_copy(g16, g_p)

            # resolvent scalars: ev = [ex | ey]; t = ev/max(ev); im = 1/(1+t)
            # re = sqrt(t)*im; both scaled by sqrt(LMBDA)
            evmax = sp.tile([1, 1], F32)
            nc.vector.tensor_reduce(evmax, ev_t, mybir.AxisListType.X,
                                    mybir.AluOpType.max)
            evrec = sp.tile([1, 1], F32)
            nc.vector.reciprocal(evrec, evmax)
            t_t = sp.tile([1, 2 * K], F32)
            nc.vector.tensor_scalar_mul(t_t, ev_t, evrec)
            tp1 = sp.tile([1, 2 * K], F32)
            nc.vector.tensor_scalar_add(tp1, t_t, 1.0)
            im_t = sp.tile([1, 2 * K], F32)
            nc.vector.reciprocal(im_t, tp1)
            sq_t = sp.tile([1, 2 * K], F32)
            nc.scalar.sqrt(sq_t, t_t)
            re_t = sp.tile([1, 2 * K], F32)
            nc.vector.tensor_mul(re_t, sq_t, im_t)
            nc.vector.tensor_scalar_mul(re_t, re_t, SQRT_LMBDA)
            nc.vector.tensor_scalar_mul(im_t, im_t, SQRT_LMBDA)

            # Newton-Schulz inverse (S symmetric PD), two-hop steps:
            #   B = 2I - S X  (DVE STT, fp16 out);  X' = X B  (X symmetric).
            # interleave(j) fills the PE-queue gaps with projection matmuls.
            def newton_inverse(mat_p, s16, tag, steps, interleave=None):
                # X0 = I / ||S||_F  (PE partition-reduce; no gpsimd library)
                prod = wp.tile([K, K], F32, tag="dot_dm", name=f"{tag}_sq")
                sqa = sp.tile([K, 1], F16, tag=f"{tag}_rs", name=f"{tag}_rs")
                with nc.allow_low_precision(reason="fp16 PE-reduce feeder"):
                    nc.vector.scalar_tensor_tensor(
                        prod, s16, 2.0 ** -8, s16,
                        op0=mybir.AluOpType.mult, op1=mybir.AluOpType.mult,
                        accum_out=sqa)
                tf_p = preduce(sqa, tag)       # ||S||_F^2 * 2^-8
                sf = sp.tile([1, 1], F32, tag=f"{tag}_sf", name=f"{tag}_sf")
                nc.scalar.sqrt(sf, tf_p)       # ||S||_F * 2^-4
                rec = sp.tile([1, 1], F32, tag=f"{tag}_rc", name=f"{tag}_rc")
                nc.vector.reciprocal(rec, sf)
                alr = sp.tile([1, 1], F16, tag=f"{tag}_al", name=f"{tag}_al")
                nc.vector.tensor_scalar_mul(alr, rec, 2.0 ** -4)  # 1/||S||_F
                al_bc = bcast(alr, tag)
                x_s = sp.tile([K, K], F16, tag=f"{tag}_x0", name=f"{tag}_x0")
                nc.vector.tensor_scalar_mul(x_s, id64, al_bc)
                it_i = 0
                for it in range(steps):
                    t1 = ps_tile([K, K])
                    nc.tensor.matmul(t1, s16, x_s)            # S X (S sym)
                    if interleave is not None:
                        interleave(it_i); it_i += 1
                    b16 = wp.tile([K, K], F16, tag=f"{tag}_b16",
                                  name=f"{tag}_b16")
                    nc.vector.scalar_tensor_tensor(
                        b16, id64, 2.0, t1,
                        op0=mybir.AluOpType.mult,
                        op1=mybir.AluOpType.subtract)         # 2I - S X
                    xn = ps_tile([K, K])
                    nc.tensor.matmul(xn, x_s, b16)            # X (2I - S X)
                    if interleave is not None:
                        interleave(it_i); it_i += 1
                    x_new = sp.tile([K, K], F16, tag=f"{tag}_x{it + 1}",
                                    name=f"{tag}_x{it + 1}")
                    if it == steps - 1:
                        nc.scalar.copy(x_new, xn)
                    else:
                        nc.vector.tensor_copy(x_new, xn)
                    x_s = x_new
                if interleave is not None:
                    interleave(1000)   # flush any remainder
                return x_s

            # ---- x projections interleaved into Newton-G's PE gaps --------
            with tc.tile_pool(name="pacc", bufs=1, space="PSUM") as pacc:
                pj_p = pacc.tile([C, 2 * K], F32)
                at_p = pj_p[:, 0:K]              # A^T  [C,K]
                byt_p = pj_p[:, K:2 * K]         # By^T [C,K]

                xprog = {"n": 0}

                def emit_xproj(upto):
                    while xprog["n"] < min(upto, NB + 1):
                        n = xprog["n"]
                        if n < NB:
                            nc.tensor.matmul(
                                at_p, xc_t[:, n, 0:C], xc_t[:, n, C:CK],
                                start=(n == 0), stop=False)
                        else:
                            nc.tensor.matmul(
                                at_p, xtl[:, 0:C], xtl[:, C:CK],
                                start=False, stop=True)
                        xprog["n"] += 1

                gi16 = newton_inverse(
                    g_p, g16, "gi", NEWTON_STEPS_G,
                    interleave=lambda j: emit_xproj(j * 6))
                emit_xproj(NB + 1)

                at16 = sp.tile([C, K], F16, tag="at16", name="at16")
                nc.vector.tensor_copy(at16, at_p)

                # S~ = Mx^T (A A^T) Mx
                sa_p = ps_tile([K, K])
                nc.tensor.matmul(sa_p, at16, at16)          # A A^T
                sa16 = wp.tile([K, K], F16, tag="sa16", name="sa16")
                nc.vector.tensor_copy(sa16, sa_p)
                h1_p = ps_tile([K, K])
                nc.tensor.matmul(h1_p, sa16, mx16)          # S_A Mx (sym)
                h16 = wp.tile([K, K], F16, tag="h16", name="h16")
                nc.vector.tensor_copy(h16, h1_p)
                st_p = ps_tile([K, K])
                nc.tensor.matmul(st_p, mx16, h16)           # Mx^T S_A Mx
                st16 = sp.tile([K, K], F16, tag="st16", name="st16")
                nc.scalar.copy(st16, st_p)

                # ---- y projections interleaved into Newton-S's gaps -------
                yprog = {"n": 0}

                def emit_yproj(upto):
                    while yprog["n"] < min(upto, NB + 1):
                        n = yprog["n"]
                        if n < NB:
                            nc.tensor.matmul(
                                byt_p, yc_t[:, n, 0:C], yc_t[:, n, C:CK],
                                start=(n == 0), stop=False)
                        else:
                            nc.tensor.matmul(
                                byt_p, ytl[:, 0:C], ytl[:, C:CK],
                                start=False, stop=True)
                        yprog["n"] += 1

                si16 = newton_inverse(
                    st_p, st16, "si", NEWTON_STEPS_S,
                    interleave=lambda j: emit_yproj((j - 3) * 6))
                emit_yproj(NB + 1)

                byt16 = sp.tile([C, K], F16, tag="byt16", name="byt16")
                nc.vector.tensor_copy(byt16, byt_p)

            # D1T[a,i] = re2[a] - re1[i]; D2T likewise from im (emitted late
            # so the tiny mask matmuls never stall the PE queue)
            d12t_s = sp.tile([K, 2 * K], F32)
            for idx, src in enumerate((re_t, im_t)):
                pa = ps_tile([K, K])
                nc.tensor.matmul(pa, src[0:1, K:2 * K], ones_row)  # v2[p]
                pb = ps_tile([K, K])
                nc.tensor.matmul(pb, ones_row, src[0:1, 0:K])      # v1[f]
                ta = wp.tile([K, K], F32, tag=f"dta{idx}", name=f"dta{idx}")
                nc.vector.tensor_copy(ta, pa)
                nc.vector.tensor_sub(d12t_s[:, idx * K:(idx + 1) * K], ta, pb)
            d12v = d12t_s[:, :].rearrange("p (a b) -> p a b", a=2)

            # ---- RHS' = G By A^T Mx (3 matmuls, G-symmetry trick) ---------
            byat_p = ps_tile([K, K])
            nc.tensor.matmul(byat_p, byt16, at16)       # By A^T
            byat16 = wp.tile([K, K], F16, tag="byat16", name="byat16")
            nc.scalar.copy(byat16, byat_p)
            s2_p = ps_tile([K, K])
            nc.tensor.matmul(s2_p, byat16, g16)         # (G ByA^T)^T (G sym)
            s2c = wp.tile([K, K], F16, tag="s2c", name="s2c")
            nc.scalar.copy(s2c, s2_p)
            rp_p = ps_tile([K, K])
            nc.tensor.matmul(rp_p, s2c, mx16)           # G ByA^T Mx

            # ------- PCG state ---------------------------------------------
            # rz_s = [r | z] fp32 (fused alpha-update); p16 fp16 direction;
            # q_s, s_s fp32; y accumulates Y^T in PSUM via matmul.
            rz_s = sp.tile([K, 2 * K], F32)
            w_s = sp.tile([K, K], F32)
            u16 = sp.tile([K, 2 * K], F16)
            p16 = sp.tile([K, K], F16)
            y_p = yp.tile([K, K], F32)
            r_sl = rz_s[:, 0:K]
            z_sl = rz_s[:, K:2 * K]
            qs_v = sp.tile([K, 2 * K], F32)   # [q | s] fused tile
            q_s = qs_v[:, 0:K]
            s_s = qs_v[:, K:2 * K]
            u16v = u16[:, :].rearrange("p (a b) -> p a b", a=2)
            z_bc = z_sl.rearrange("p (o b) -> p o b", o=1).broadcast_to(
                [K, 2, K])

            nc.vector.tensor_copy(r_sl, rp_p)
            r16 = wp.tile([K, K], F16, tag="x16", name="r16i")
            nc.scalar.copy(r16, rp_p)

            def precond_psum(x16, tag):
                """P^-1 x in PSUM via (Gi x)^T = mm(lhsT=x16, rhs=Gi)."""
                ut_p = ps_tile([K, K])
                nc.tensor.matmul(ut_p, x16, gi16)
                ut16 = wp.tile([K, K], F16, tag=f"{tag}_ut", name=f"{tag}_ut")
                nc.scalar.copy(ut16, ut_p)
                v_p = ps_tile([K, K])
                nc.tensor.matmul(v_p, ut16, si16)
                return v_p

            def matvec_z(z16, tag):
                """w = M z -> w_s; w16 returned for the precond."""
                nc.vector.tensor_mul(u16v, d12v, z_bc)   # [D1T*z | D2T*z]
                gzt_p = ps_tile([K, K])
                nc.tensor.matmul(gzt_p, z16, g16)         # (G z)^T
                gzt16 = wp.tile([K, K], F16, tag="mv_gzt", name="mv_gzt")
                nc.scalar.copy(gzt16, gzt_p)
                gu_p = ps_tile([K, 3 * K])
                nc.tensor.matmul(gu_p[:, 0:2 * K], g16, u16)   # G u (both)
                nc.tensor.matmul(gu_p[:, 2 * K:3 * K], gzt16, st16)  # (Gz)S~
                mm_s = wp.tile([K, 2 * K], F32, tag="mv_mm", name="mv_mm")
                nc.vector.tensor_mul(mm_s, d12t_s, gu_p[:, 0:2 * K])  # mask
                a1_s = wp.tile([K, K], F32, tag="mv_a1", name="mv_a1")
                nc.vector.tensor_add(a1_s, mm_s[:, 0:K], mm_s[:, K:2 * K])
                nc.vector.tensor_add(w_s, a1_s, gu_p[:, 2 * K:3 * K])
                w16 = wp.tile([K, K], F16, tag="x16", name=f"{tag}_w16")
                nc.scalar.copy(w16, w_s)
                return w16

            def dot_b(a_ap, b_ap, tag):
                """<a,b> as [1,1] PSUM (DVE accumulate + PE reduce)."""
                prod = wp.tile([K, K], F32, tag="dot_dm", name="dot_dm")
                acc = wp.tile([K, 1], F16, tag=f"{tag}_acc",
                              name=f"{tag}_acc")
                with nc.allow_low_precision(reason="fp16 PE-reduce feeder"):
                    nc.vector.scalar_tensor_tensor(
                        prod, a_ap, 2.0 ** -8, b_ap,
                        op0=mybir.AluOpType.mult, op1=mybir.AluOpType.mult,
                        accum_out=acc)
                return preduce(acc, tag)

            # init: z = P^-1 r; p = z; w = Mz; v = P^-1 w; q = w, s = v
            z0_p = precond_psum(r16, "pcz")
            nc.vector.tensor_copy(z_sl, z0_p)
            nc.scalar.copy(p16, z0_p)
            z16 = wp.tile([K, K], F16, tag="z16", name="z16i")
            nc.scalar.copy(z16, z0_p)
            rz_p = dot_b(r_sl, z_sl, "rz")
            rz11 = wp.tile([1, 1], F32, tag="rz11", name="rz11")
            nc.vector.tensor_copy(rz11, rz_p)
            rzrec = wp.tile([1, 1], F32, tag="rzrec", name="rzrec")
            nc.vector.reciprocal(rzrec, rz_p)
            rzneg = wp.tile([1, 1], F32, tag="rzneg", name="rzneg")
            nc.vector.tensor_scalar_mul(rzneg, rz_p, -1.0)
            w16 = matvec_z(z16, "init")
            nc.vector.tensor_copy(q_s, w_s)
            v_p = precond_psum(w16, "pcv")
            nc.vector.tensor_copy(s_s, v_p)

            for it in range(N_ITERS):
                # ---- alpha = rz/<p,q>; fused [r|z] -= alpha [q|s] ----
                pq_p = dot_b(p16, q_s, "pq")
                pqr = wp.tile([1, 1], F32, tag="pqr", name="pqr")
                nc.vector.reciprocal(pqr, pq_p)
                def emit_y_update():
                    # y^T += alpha p^T (PSUM accumulate, off critical path;
                    # must be emitted BEFORE the p16 beta-update)
                    al = wp.tile([1, 1], F16, tag="al", name="al")
                    nc.vector.tensor_mul(al, rz11, pqr)
                    al_bc = bcast(al, "al")
                    al_s = wp.tile([K, 1], F32, tag="al_s", name="al_s")
                    nc.scalar.copy(al_s, al_bc)
                    ida = wp.tile([K, K], F16, tag="ida", name="ida")
                    nc.scalar.mul(ida, id16, al_s)
                    nc.tensor.matmul(y_p, p16, ida,
                                     start=(it == 0),
                                     stop=(it == N_ITERS - 1))

                if it < N_ITERS - 1:
                    an = wp.tile([1, 1], F16, tag="an", name="an")
                    nc.vector.tensor_mul(an, rzneg, pqr)
                    an_bc = bcast(an, "an")
                    nc.vector.scalar_tensor_tensor(
                        rz_s, qs_v, an_bc, rz_s,
                        op0=mybir.AluOpType.mult, op1=mybir.AluOpType.add)
                    z16 = wp.tile([K, K], F16, tag="z16", name=f"z16_{it}")
                    nc.scalar.copy(z16, z_sl)

                    # ---- rz_new; matvec + precond for next q,s ----
                    rznew_p = dot_b(r_sl, z_sl, "rz")
                    bt = wp.tile([1, 1], F16, tag="bt", name="bt")
                    nc.vector.tensor_mul(bt, rznew_p, rzrec)
                    bt_bc = bcast(bt, "bt")
                    w16 = matvec_z(z16, f"i{it}")
                    emit_y_update()   # uses this iteration's rz11 (alpha)
                    rz11 = wp.tile([1, 1], F32, tag="rz11", name="rz11")
                    nc.vector.tensor_copy(rz11, rznew_p)
                    rzrec = wp.tile([1, 1], F32, tag="rzrec", name="rzrec")
                    nc.vector.reciprocal(rzrec, rznew_p)
                    rzneg = wp.tile([1, 1], F32, tag="rzneg", name="rzneg")
                    nc.vector.tensor_scalar_mul(rzneg, rznew_p, -1.0)
                    if it < N_ITERS - 2:
                        v_s = wp.tile([K, K], F32, tag="v_s", name="v_s")
                        v_p = precond_psum(w16, "pcv")
                        nc.scalar.copy(v_s, v_p)
                    nc.vector.scalar_tensor_tensor(
                        p16, p16, bt_bc, z_sl,
                        op0=mybir.AluOpType.mult, op1=mybir.AluOpType.add)
                    nc.vector.scalar_tensor_tensor(
                        q_s, q_s, bt_bc, w_s,
                        op0=mybir.AluOpType.mult, op1=mybir.AluOpType.add)
                    if it < N_ITERS - 2:
                        nc.vector.scalar_tensor_tensor(
                            s_s, s_s, bt_bc, v_s,
                            op0=mybir.AluOpType.mult, op1=mybir.AluOpType.add)
                else:
                    emit_y_update()
                    break

            # -------- output: C = Y Mx^T  (y_p holds Y^T) ------------------
            y16 = wp.tile([K, K], F16, tag="y16", name="y16")
            nc.scalar.copy(y16, y_p)
            c_p = ps_tile([K, K])
            nc.tensor.matmul(c_p, y16, mxT16)           # (Y^T)^T Mx^T
            c_s = wp.tile([K, K], F32, tag="c_s", name="c_s")
            nc.vector.tensor_copy(c_s, c_p)
            nc.sync.dma_start(out_d[:, :], c_s)

    nc.finalize()
    return nc


def get_program(shard: bool = False):
    if shard not in _PROGRAM_CACHE:
        _PROGRAM_CACHE[shard] = build_program(shard)
    return _PROGRAM_CACHE[shard]


def make_in_maps(inputs, shard: bool = False):
    fx = np.asarray(inputs["feat_x"], np.float32)[0]
    fy = np.asarray(inputs["feat_y"], np.float32)[0]
    pxT = np.asarray(inputs["evecs_trans_x"], np.float32)[0].T
    pyT = np.asarray(inputs["evecs_trans_y"], np.float32)[0].T
    xc = np.ascontiguousarray(
        np.concatenate([fx, pxT], axis=1)).astype(np.float16)
    yc = np.ascontiguousarray(
        np.concatenate([fy, pyT], axis=1)).astype(np.float16)
    mx = np.asarray(inputs["sqrtMk_x"], np.float32)[0]
    my = np.asarray(inputs["sqrtMk_y"], np.float32)[0]
    small = np.ascontiguousarray(np.concatenate(
        [mx, my, mx.T, np.eye(64, dtype=np.float32)], axis=1))
    ev = np.ascontiguousarray(np.concatenate([
        np.asarray(inputs["evals_x"], np.float32)[0],
        np.asarray(inputs["evals_y"], np.float32)[0],
    ])[None, :])
    m = {"xc": xc, "yc": yc, "small": small, "ev": ev}
    return [dict(m) for _ in range(N_CORES)]


def kernel(**inputs) -> np.ndarray:
    nc = get_program(SHARD)
    in_maps = make_in_maps(inputs, SHARD)
    res = run_bass_kernel_spmd(nc, in_maps, core_ids=list(range(N_CORES)))
    out = np.asarray(res.results[0]["out"], dtype=np.float32)
    return out[None]


# revision 28
# speedup vs baseline: 1.0596x; 1.0003x over previous
"""Trainium2 Bass kernel for nn_ExpandedResolventFMNet.

Mathematical reformulation (validated in fp64/fp16 against the jax reference):

The reference builds kron(A.T, My) [8192x4096], its Gram [4096^2], resolvent
kron masks, and solves a dense 4096x4096 system.  All of that collapses:

  first        = kron(A A^T, G),              G = My^T My
  second       = kron-sum of 64x64 factors; with X = Mx W the full system is
  M(W)         = S~ W G + LMBDA * sum_d Dd*( (Dd*W) G ) = R~    (* = Hadamard)
  S~           = Mx^T (A A^T) Mx
  R~           = G By A^T Mx,   By = Py fy
  Dd           = resolvent-mask difference matrices (64x64)
  output C     = (Mx W)^T

The device runs the transposed system Y = W^T:

  M'(Y) = G Y S~ + sum_d DdT * (G (DdT * Y)),   C = Y Mx^T

solved by PCG with the exact-kron preconditioner P^-1 x = Gi x Si, where
Gi, Si come from on-device Newton-Schulz iteration (two-hop steps:
X' = X (2I - S X)).  G's symmetry makes every matmul transpose-free, and
Y^T is accumulated in PSUM via matmul against an alpha-scaled identity so
the output needs no transpose.

Fully unsharded: every core redundantly computes the whole answer, so there
are no collectives (SPMD launch skew made the barrier + two AllReduce cost
~64us on the measured core).  All matmuls run in fp16 (single-pass, 1
cycle/row vs fp32's split LOW_HIGH 2-pass) with fp32 PSUM accumulation; CG
state stays fp32 except the search direction p (fp16).  Inputs are cast to
fp16 on the host (halves HBM traffic).  The four big tensors stream through
the gpsimd SWDGE queue, which stripes descriptors over all 16 SDMA engines
(the HWDGE rings only get 5); queue FIFO order gives the x-side strict
priority.  Each partition's data is contiguous in DRAM (125 descriptors
per tensor).
"""

import numpy as np

import concourse.bacc as bacc
import concourse.mybir as mybir
from concourse.bass_utils import run_bass_kernel_spmd
from concourse.tile import TileContext

F32 = mybir.dt.float32
F16 = mybir.dt.float16
F32R = mybir.dt.float32r
K = 64          # spectral basis size
C = 128         # feature channels
V = 5000        # vertices
P = 128         # DMA partition rows
NB = 39         # full contraction chunks (V = P * NB + TAIL)
TAIL = 8        # leftover rows handled by one small matmul
N_CORES = 8
N_ITERS = 6
NEWTON_STEPS_S = 7
NEWTON_STEPS_G = 4
SQRT_LMBDA = 10.0

SHARD = False   # kept for test.py compat; only the unsharded path exists

_PROGRAM_CACHE = {}


def build_program(shard: bool):
    nc = bacc.Bacc("TRN2", num_devices=N_CORES)

    CK = C + K      # concatenated row: [fx | pxT]
    xc_d = nc.dram_tensor("xc", [V, CK], F16, kind="ExternalInput")
    yc_d = nc.dram_tensor("yc", [V, CK], F16, kind="ExternalInput")
    # mx|my|mxT|I [64, 4*64] fp32 (host-concatenated; I avoids the gpsimd
    # affine_select library swap that delayed partition_all_reduce ~13us)
    small_d = nc.dram_tensor("small", [K, 4 * K], F32, kind="ExternalInput")
    ev_d = nc.dram_tensor("ev", [1, 2 * K], F32, kind="ExternalInput")
    out_d = nc.dram_tensor("out", [K, K], F32, kind="ExternalOutput")

    xc_ap = xc_d[0:P * NB, :].rearrange("(p n) c -> p n c", p=P)
    yc_ap = yc_d[0:P * NB, :].rearrange("(p n) c -> p n c", p=P)

    with TileContext(nc) as tc:
        with (
            tc.tile_pool(name="big", bufs=1) as bp,
            tc.tile_pool(name="persist", bufs=1) as sp,
            tc.tile_pool(name="work", bufs=2) as wp,
            tc.tile_pool(name="psum", bufs=2, space="PSUM") as pp,
            tc.tile_pool(name="yacc", bufs=1, space="PSUM") as yp,
            tc.tile_pool(name="scal", bufs=2, space="PSUM") as scp,
        ):
            _sc_state = {"i": 0}

            def sc_tile(shape):
                i = _sc_state["i"]
                _sc_state["i"] += 1
                return scp.tile(shape, F32, tag="sc", name=f"sct{i}")
            _ps_state = {"i": 0}

            def ps_tile(shape):
                i = _ps_state["i"]
                _ps_state["i"] += 1
                return pp.tile(shape, F32, tag=f"ps{i % 2}", name=f"pst{i}")

            # ---------------- input DMA ------------------------------------
            # smalls ride the (otherwise idle) HWDGE queues; the four big
            # tensors stream through gpsimd SWDGE in x-first FIFO order.
            small_t = sp.tile([K, 4 * K], F32)
            ev_t = sp.tile([1, 2 * K], F32)
            xc_t = bp.tile([P, NB, CK], F16)
            yc_t = bp.tile([P, NB, CK], F16)
            xtl = sp.tile([TAIL, CK], F16)
            ytl = sp.tile([TAIL, CK], F16)
            H = NB // 2
            nc.sync.dma_start(small_t, small_d[:, :])
            nc.sync.dma_start(ev_t, ev_d[:, :])
            nc.sync.dma_start(xtl, xc_d[P * NB:V, :])
            nc.sync.dma_start(ytl, yc_d[P * NB:V, :])
            nc.gpsimd.dma_start(xc_t[:, 0:H, :], xc_ap[:, 0:H, :])
            nc.gpsimd.dma_start(xc_t[:, H:NB, :], xc_ap[:, H:NB, :])
            nc.gpsimd.dma_start(yc_t[:, 0:H, :], yc_ap[:, 0:H, :])
            nc.gpsimd.dma_start(yc_t[:, H:NB, :], yc_ap[:, H:NB, :])

            mx_s = small_t[:, 0:K]
            my_s = small_t[:, K:2 * K]
            mxT_s = small_t[:, 2 * K:3 * K]
            id64 = small_t[:, 3 * K:4 * K]

            # fp16 copies of the small matrices (scalar engine)
            m16 = sp.tile([K, 3 * K], F16)
            nc.scalar.copy(m16[:, 0:K], mx_s)
            nc.scalar.copy(m16[:, K:2 * K], my_s)
            nc.scalar.copy(m16[:, 2 * K:3 * K], mxT_s)
            mx16 = m16[:, 0:K]
            my16 = m16[:, K:2 * K]
            mxT16 = m16[:, 2 * K:3 * K]

            id16 = sp.tile([K, K], F16)
            nc.scalar.copy(id16, id64)
            ones_row = sp.tile([1, K], F32)
            nc.vector.memset(ones_row, 1.0)
            ones_row16 = sp.tile([1, K], F16)
            nc.scalar.copy(ones_row16, ones_row)
            ones_col = sp.tile([K, 1], F32)
            nc.vector.memset(ones_col, 1.0)
            ones_col16 = sp.tile([K, 1], F16)
            nc.scalar.copy(ones_col16, ones_col)

            def preduce(acc16, tag):
                """sum over partitions: fp16 [K,1] -> [1,1] PSUM via PE
                matmul (avoids the gpsimd custom-op library + ~12us load).
                Feeder values are pre-scaled by 2^-8 to fit fp16; the scale
                cancels in every alpha/beta ratio."""
                t_p = sc_tile([1, 1])
                nc.tensor.matmul(t_p, acc16, ones_col16[:, :])
                return t_p

            def bcast(s11_16, tag):
                """fp16 [1,1] SBUF -> [K,1] PSUM broadcast via PE matmul"""
                b_p = sc_tile([K, 1])
                nc.tensor.matmul(b_p, ones_row16[:, :], s11_16)
                return b_p

            # ---------------- G = My^T My (early; data lands first) --------
            g_p = ps_tile([K, K])
            nc.tensor.matmul(g_p, my16, my16)
            g16 = sp.tile([K, K], F16, tag="g16", name="g16")
            nc.vector.tensor_copy(g16, g_p)

            # resolvent scalars: ev = [ex | ey]; t = ev/max(ev); im = 1/(1+t)
            # re = sqrt(t)*im; both scaled by sqrt(LMBDA)
            evmax = sp.tile([1, 1], F32)
            nc.vector.tensor_reduce(evmax, ev_t, mybir.AxisListType.X,
                                    mybir.AluOpType.max)
            evrec = sp.tile([1, 1], F32)
            nc.vector.reciprocal(evrec, evmax)
            t_t = sp.tile([1, 2 * K], F32)
            nc.vector.tensor_scalar_mul(t_t, ev_t, evrec)
            tp1 = sp.tile([1, 2 * K], F32)
            nc.vector.tensor_scalar_add(tp1, t_t, 1.0)
            im_t = sp.tile([1, 2 * K], F32)
            nc.vector.reciprocal(im_t, tp1)
            sq_t = sp.tile([1, 2 * K], F32)
            nc.scalar.sqrt(sq_t, t_t)
            re_t = sp.tile([1, 2 * K], F32)
            nc.vector.tensor_mul(re_t, sq_t, im_t)
            nc.vector.tensor_scalar_mul(re_t, re_t, SQRT_LMBDA)
            nc.vector.tensor_scalar_mul(im_t, im_t, SQRT_LMBDA)

            # Newton-Schulz inverse (S symmetric PD), two-hop steps:
            #   B = 2I - S X  (DVE STT, fp16 out);  X' = X B  (X symmetric).
            # interleave(j) fills the PE-queue gaps with projection matmuls.
            def newton_inverse(mat_p, s16, tag, steps, interleave=None):
                # X0 = I / ||S||_F  (PE partition-reduce; no gpsimd library)
                prod = wp.tile([K, K], F32, tag="dot_dm", name=f"{tag}_sq")
                sqa = sp.tile([K, 1], F16, tag=f"{tag}_rs", name=f"{tag}_rs")
                with nc.allow_low_precision(reason="fp16 PE-reduce feeder"):
                    nc.vector.scalar_tensor_tensor(
                        prod, s16, 2.0 ** -8, s16,
                        op0=mybir.AluOpType.mult, op1=mybir.AluOpType.mult,
                        accum_out=sqa)
                tf_p = preduce(sqa, tag)       # ||S||_F^2 * 2^-8
                sf = sp.tile([1, 1], F32, tag=f"{tag}_sf", name=f"{tag}_sf")
                nc.scalar.sqrt(sf, tf_p)       # ||S||_F * 2^-4
                rec = sp.tile([1, 1], F32, tag=f"{tag}_rc", name=f"{tag}_rc")
                nc.vector.reciprocal(rec, sf)
                alr = sp.tile([1, 1], F16, tag=f"{tag}_al", name=f"{tag}_al")
                nc.vector.tensor_scalar_mul(alr, rec, 2.0 ** -4)  # 1/||S||_F
                al_bc = bcast(alr, tag)
                x_s = sp.tile([K, K], F16, tag=f"{tag}_x0", name=f"{tag}_x0")
                nc.vector.tensor_scalar_mul(x_s, id64, al_bc)
                it_i = 0
                for it in range(steps):
                    t1 = ps_tile([K, K])
                    nc.tensor.matmul(t1, s16, x_s)            # S X (S sym)
                    if interleave is not None:
                        interleave(it_i); it_i += 1
                    b16 = wp.tile([K, K], F16, tag=f"{tag}_b16",
                                  name=f"{tag}_b16")
                    nc.vector.scalar_tensor_tensor(
                        b16, id64, 2.0, t1,
                        op0=mybir.AluOpType.mult,
                        op1=mybir.AluOpType.subtract)         # 2I - S X
                    xn = ps_tile([K, K])
                    nc.tensor.matmul(xn, x_s, b16)            # X (2I - S X)
                    if interleave is not None:
                        interleave(it_i); it_i += 1
                    x_new = sp.tile([K, K], F16, tag=f"{tag}_x{it + 1}",
                                    name=f"{tag}_x{it + 1}")
                    if it == steps - 1:
                        nc.scalar.copy(x_new, xn)
                    else:
                        nc.vector.tensor_copy(x_new, xn)
                    x_s = x_new
                if interleave is not None:
                    interleave(1000)   # flush any remainder
                return x_s

            # ---- x projections interleaved into Newton-G's PE gaps --------
            with tc.tile_pool(name="pacc", bufs=1, space="PSUM") as pacc:
                pj_p = pacc.tile([C, 2 * K], F32)
                at_p = pj_p[:, 0:K]              # A^T  [C,K]
                byt_p = pj_p[:, K:2 * K]         # By^T [C,K]

                xprog = {"n": 0}

                def emit_xproj(upto):
                    while xprog["n"] < min(upto, NB + 1):
                        n = xprog["n"]
                        if n < NB:
                            nc.tensor.matmul(
                                at_p, xc_t[:, n, 0:C], xc_t[:, n, C:CK],
                                start=(n == 0), stop=False)
                        else:
                            nc.tensor.matmul(
                                at_p, xtl[:, 0:C], xtl[:, C:CK],
                                start=False, stop=True)
                        xprog["n"] += 1

                gi16 = newton_inverse(
                    g_p, g16, "gi", NEWTON_STEPS_G,
                    interleave=lambda j: emit_xproj(j * 6))
                emit_xproj(NB + 1)

                at16 = sp.tile([C, K], F16, tag="at16", name="at16")
                nc.vector.tensor_copy(at16, at_p)

                # S~ = Mx^T (A A^T) Mx
                sa_p = ps_tile([K, K])
                nc.tensor.matmul(sa_p, at16, at16)          # A A^T
                sa16 = wp.tile([K, K], F16, tag="sa16", name="sa16")
                nc.vector.tensor_copy(sa16, sa_p)
                h1_p = ps_tile([K, K])
                nc.tensor.matmul(h1_p, sa16, mx16)          # S_A Mx (sym)
                h16 = wp.tile([K, K], F16, tag="h16", name="h16")
                nc.vector.tensor_copy(h16, h1_p)
                st_p = ps_tile([K, K])
                nc.tensor.matmul(st_p, mx16, h16)           # Mx^T S_A Mx
                st16 = sp.tile([K, K], F16, tag="st16", name="st16")
                nc.scalar.copy(st16, st_p)

                # ---- y projections interleaved into Newton-S's gaps -------
                yprog = {"n": 0}

                def emit_yproj(upto):
                    while yprog["n"] < min(upto, NB + 1):
                        n = yprog["n"]
                        if n < NB:
                            nc.tensor.matmul(
                                byt_p, yc_t[:, n, 0:C], yc_t[:, n, C:CK],
                                start=(n == 0), stop=False)
                        else:
                            nc.tensor.matmul(
                                byt_p, ytl[:, 0:C], ytl[:, C:CK],
                                start=False, stop=True)
                        yprog["n"] += 1

                si16 = newton_inverse(
                    st_p, st16, "si", NEWTON_STEPS_S,
                    interleave=lambda j: emit_yproj((j - 3) * 6))
                emit_yproj(NB + 1)

                byt16 = sp.tile([C, K], F16, tag="byt16", name="byt16")
                nc.vector.tensor_copy(byt16, byt_p)

            # D1T[a,i] = re2[a] - re1[i]; D2T likewise from im (emitted late
            # so the tiny mask matmuls never stall the PE queue)
            d12t_s = sp.tile([K, 2 * K], F32)
            for idx, src in enumerate((re_t, im_t)):
                pa = ps_tile([K, K])
                nc.tensor.matmul(pa, src[0:1, K:2 * K], ones_row)  # v2[p]
                pb = ps_tile([K, K])
                nc.tensor.matmul(pb, ones_row, src[0:1, 0:K])      # v1[f]
                ta = wp.tile([K, K], F32, tag=f"dta{idx}", name=f"dta{idx}")
                nc.vector.tensor_copy(ta, pa)
                nc.vector.tensor_sub(d12t_s[:, idx * K:(idx + 1) * K], ta, pb)
            d12v = d12t_s[:, :].rearrange("p (a b) -> p a b", a=2)

            # ---- RHS' = G By A^T Mx (3 matmuls, G-symmetry trick) ---------
            byat_p = ps_tile([K, K])
            nc.tensor.matmul(byat_p, byt16, at16)       # By A^T
            byat16 = wp.tile([K, K], F16, tag="byat16", name="byat16")
            nc.scalar.copy(byat16, byat_p)
            s2_p = ps_tile([K, K])
            nc.tensor.matmul(s2_p, byat16, g16)         # (G ByA^T)^T (G sym)
            s2c = wp.tile([K, K], F16, tag="s2c", name="s2c")
            nc.scalar.copy(s2c, s2_p)
            rp_p = ps_tile([K, K])
            nc.tensor.matmul(rp_p, s2c, mx16)           # G ByA^T Mx

            # ------- PCG state ---------------------------------------------
            # rz_s = [r | z] fp32 (fused alpha-update); p16 fp16 direction;
            # q_s, s_s fp32; y accumulates Y^T in PSUM via matmul.
            rz_s = sp.tile([K, 2 * K], F32)
            w_s = sp.tile([K, K], F32)
            u16 = sp.tile([K, 2 * K], F16)
            p16 = sp.tile([K, K], F16)
            y_p = yp.tile([K, K], F32)
            r_sl = rz_s[:, 0:K]
            z_sl = rz_s[:, K:2 * K]
            qs_v = sp.tile([K, 2 * K], F32)   # [q | s] fused tile
            q_s = qs_v[:, 0:K]
            s_s = qs_v[:, K:2 * K]
            u16v = u16[:, :].rearrange("p (a b) -> p a b", a=2)
            z_bc = z_sl.rearrange("p (o b) -> p o b", o=1).broadcast_to(
                [K, 2, K])

            nc.vector.tensor_copy(r_sl, rp_p)
            r16 = wp.tile([K, K], F16, tag="x16", name="r16i")
            nc.scalar.copy(r16, rp_p)

            def precond_psum(x16, tag):
                """P^-1 x in PSUM via (Gi x)^T = mm(lhsT=x16, rhs=Gi)."""
                ut_p = ps_tile([K, K])
                nc.tensor.matmul(ut_p, x16, gi16)
                ut16 = wp.tile([K, K], F16, tag=f"{tag}_ut", name=f"{tag}_ut")
                nc.scalar.copy(ut16, ut_p)
                v_p = ps_tile([K, K])
                nc.tensor.matmul(v_p, ut16, si16)
                return v_p

            def matvec_z(z16, tag):
                """w = M z -> w_s; w16 returned for the precond."""
                nc.vector.tensor_mul(u16v, d12v, z_bc)   # [D1T*z | D2T*z]
                gzt_p = ps_tile([K, K])
                nc.tensor.matmul(gzt_p, z16, g16)         # (G z)^T
                gzt16 = wp.tile([K, K], F16, tag="mv_gzt", name="mv_gzt")
                nc.vector.tensor_copy(gzt16, gzt_p)
                gu_p = ps_tile([K, 3 * K])
                nc.tensor.matmul(gu_p[:, 0:2 * K], g16, u16)   # G u (both)
                nc.tensor.matmul(gu_p[:, 2 * K:3 * K], gzt16, st16)  # (Gz)S~
                mm_s = wp.tile([K, 2 * K], F32, tag="mv_mm", name="mv_mm")
                nc.vector.tensor_mul(mm_s, d12t_s, gu_p[:, 0:2 * K])  # mask
                a1_s = wp.tile([K, K], F32, tag="mv_a1", name="mv_a1")
                nc.vector.tensor_add(a1_s, mm_s[:, 0:K], mm_s[:, K:2 * K])
                nc.vector.tensor_add(w_s, a1_s, gu_p[:, 2 * K:3 * K])
                w16 = wp.tile([K, K], F16, tag="x16", name=f"{tag}_w16")
                nc.scalar.copy(w16, w_s)
                return w16

            def dot_b(a_ap, b_ap, tag):
                """<a,b> as [1,1] PSUM (DVE accumulate + PE reduce)."""
                prod = wp.tile([K, K], F32, tag="dot_dm", name="dot_dm")
                acc = wp.tile([K, 1], F16, tag=f"{tag}_acc",
                              name=f"{tag}_acc")
                with nc.allow_low_precision(reason="fp16 PE-reduce feeder"):
                    nc.vector.scalar_tensor_tensor(
                        prod, a_ap, 2.0 ** -8, b_ap,
                        op0=mybir.AluOpType.mult, op1=mybir.AluOpType.mult,
                        accum_out=acc)
                return preduce(acc, tag)

            # init: z = P^-1 r; p = z; w = Mz; v = P^-1 w; q = w, s = v
            z0_p = precond_psum(r16, "pcz")
            nc.vector.tensor_copy(z_sl, z0_p)
            nc.scalar.copy(p16, z0_p)
            z16 = wp.tile([K, K], F16, tag="z16", name="z16i")
            nc.scalar.copy(z16, z0_p)
            rz_p = dot_b(r_sl, z_sl, "rz")
            rz11 = wp.tile([1, 1], F32, tag="rz11", name="rz11")
            nc.vector.tensor_copy(rz11, rz_p)
            rzrec = wp.tile([1, 1], F32, tag="rzrec", name="rzrec")
            nc.vector.reciprocal(rzrec, rz_p)
            rzneg = wp.tile([1, 1], F32, tag="rzneg", name="rzneg")
            nc.vector.tensor_scalar_mul(rzneg, rz_p, -1.0)
            w16 = matvec_z(z16, "init")
            nc.vector.tensor_copy(q_s, w_s)
            v_p = precond_psum(w16, "pcv")
            nc.vector.tensor_copy(s_s, v_p)

            for it in range(N_ITERS):
                # ---- alpha = rz/<p,q>; fused [r|z] -= alpha [q|s] ----
                pq_p = dot_b(p16, q_s, "pq")
                pqr = wp.tile([1, 1], F32, tag="pqr", name="pqr")
                nc.vector.reciprocal(pqr, pq_p)
                def emit_y_update():
                    # y^T += alpha p^T (PSUM accumulate, off critical path;
                    # must be emitted BEFORE the p16 beta-update)
                    al = wp.tile([1, 1], F16, tag="al", name="al")
                    nc.vector.tensor_mul(al, rz11, pqr)
                    al_bc = bcast(al, "al")
                    al_s = wp.tile([K, 1], F32, tag="al_s", name="al_s")
                    nc.scalar.copy(al_s, al_bc)
                    ida = wp.tile([K, K], F16, tag="ida", name="ida")
                    nc.scalar.mul(ida, id16, al_s)
                    nc.tensor.matmul(y_p, p16, ida,
                                     start=(it == 0),
                                     stop=(it == N_ITERS - 1))

                if it < N_ITERS - 1:
                    an = wp.tile([1, 1], F16, tag="an", name="an")
                    nc.vector.tensor_mul(an, rzneg, pqr)
                    an_bc = bcast(an, "an")
                    nc.vector.scalar_tensor_tensor(
                        rz_s, qs_v, an_bc, rz_s,
                        op0=mybir.AluOpType.mult, op1=mybir.AluOpType.add)
                    z16 = wp.tile([K, K], F16, tag="z16", name=f"z16_{it}")
                    nc.vector.tensor_copy(z16, z_sl)

                    # ---- rz_new; matvec + precond for next q,s ----
                    rznew_p = dot_b(r_sl, z_sl, "rz")
                    bt = wp.tile([1, 1], F16, tag="bt", name="bt")
                    nc.vector.tensor_mul(bt, rznew_p, rzrec)
                    bt_bc = bcast(bt, "bt")
                    w16 = matvec_z(z16, f"i{it}")
                    emit_y_update()   # uses this iteration's rz11 (alpha)
                    rz11 = wp.tile([1, 1], F32, tag="rz11", name="rz11")
                    nc.vector.tensor_copy(rz11, rznew_p)
                    rzrec = wp.tile([1, 1], F32, tag="rzrec", name="rzrec")
                    nc.vector.reciprocal(rzrec, rznew_p)
                    rzneg = wp.tile([1, 1], F32, tag="rzneg", name="rzneg")
                    nc.vector.tensor_scalar_mul(rzneg, rznew_p, -1.0)
                    if it < N_ITERS - 2:
                        v_s = wp.tile([K, K], F32, tag="v_s", name="v_s")
                        v_p = precond_psum(w16, "pcv")
                        nc.scalar.copy(v_s, v_p)
                    nc.vector.scalar_tensor_tensor(
                        p16, p16, bt_bc, z_sl,
                        op0=mybir.AluOpType.mult, op1=mybir.AluOpType.add)
                    nc.vector.scalar_tensor_tensor(
                        q_s, q_s, bt_bc, w_s,
                        op0=mybir.AluOpType.mult, op1=mybir.AluOpType.add)
                    if it < N_ITERS - 2:
                        nc.vector.scalar_tensor_tensor(
                            s_s, s_s, bt_bc, v_s,
                            op0=mybir.AluOpType.mult, op1=mybir.AluOpType.add)
                else:
                    emit_y_update()
                    break

            # -------- output: C = Y Mx^T  (y_p holds Y^T) ------------------
            y16 = wp.tile([K, K], F16, tag="y16", name="y16")
            nc.scalar.copy(y16, y_p)
            c_p = ps_tile([K, K])
            nc.tensor.matmul(c_p, y16, mxT16)           # (Y^T)^T Mx^T
            c_s = wp.tile([K, K], F32, tag="c_s", name="c_s")
            nc.vector.tensor_copy(c_s, c_p)
            nc.sync.dma_start(out_d[:, :], c_s)

    nc.finalize()
    return nc


def get_program(shard: bool = False):
    if shard not in _PROGRAM_CACHE:
        _PROGRAM_CACHE[shard] = build_program(shard)
    return _PROGRAM_CACHE[shard]


def make_in_maps(inputs, shard: bool = False):
    fx = np.asarray(inputs["feat_x"], np.float32)[0]
    fy = np.asarray(inputs["feat_y"], np.float32)[0]
    pxT = np.asarray(inputs["evecs_trans_x"], np.float32)[0].T
    pyT = np.asarray(inputs["evecs_trans_y"], np.float32)[0].T
    xc = np.ascontiguousarray(
        np.concatenate([fx, pxT], axis=1)).astype(np.float16)
    yc = np.ascontiguousarray(
        np.concatenate([fy, pyT], axis=1)).astype(np.float16)
    mx = np.asarray(inputs["sqrtMk_x"], np.float32)[0]
    my = np.asarray(inputs["sqrtMk_y"], np.float32)[0]
    small = np.ascontiguousarray(np.concatenate(
        [mx, my, mx.T, np.eye(64, dtype=np.float32)], axis=1))
    ev = np.ascontiguousarray(np.concatenate([
        np.asarray(inputs["evals_x"], np.float32)[0],
        np.asarray(inputs["evals_y"], np.float32)[0],
    ])[None, :])
    m = {"xc": xc, "yc": yc, "small": small, "ev": ev}
    return [dict(m) for _ in range(N_CORES)]


def kernel(**inputs) -> np.ndarray:
    nc = get_program(SHARD)
    in_maps = make_in_maps(inputs, SHARD)
    res = run_bass_kernel_spmd(nc, in_maps, core_ids=list(range(N_CORES)))
    out = np.asarray(res.results[0]["out"], dtype=np.float32)
    return out[None]


# revision 29
# speedup vs baseline: 1.1226x; 1.0594x over previous
"""Trainium2 Bass kernel for nn_ExpandedResolventFMNet.

Mathematical reformulation (validated in fp64/fp16 against the jax reference):

The reference builds kron(A.T, My) [8192x4096], its Gram [4096^2], resolvent
kron masks, and solves a dense 4096x4096 system.  All of that collapses:

  first        = kron(A A^T, G),              G = My^T My
  second       = kron-sum of 64x64 factors; with X = Mx W the full system is
  M(W)         = S~ W G + LMBDA * sum_d Dd*( (Dd*W) G ) = R~    (* = Hadamard)
  S~           = Mx^T (A A^T) Mx
  R~           = G By A^T Mx,   By = Py fy
  Dd           = resolvent-mask difference matrices (64x64)
  output C     = (Mx W)^T

The device runs the transposed system Y = W^T:

  M'(Y) = G Y S~ + sum_d DdT * (G (DdT * Y)),   C = Y Mx^T

solved by PCG with the exact-kron preconditioner P^-1 x = Gi x Si, where
Gi, Si come from on-device Newton-Schulz iteration (two-hop steps:
X' = X (2I - S X)).  G's symmetry makes every matmul transpose-free, and
Y^T is accumulated in PSUM via matmul against an alpha-scaled identity so
the output needs no transpose.

Fully unsharded: every core redundantly computes the whole answer, so there
are no collectives (SPMD launch skew made the barrier + two AllReduce cost
~64us on the measured core).  All matmuls run in fp16 (single-pass, 1
cycle/row vs fp32's split LOW_HIGH 2-pass) with fp32 PSUM accumulation; CG
state stays fp32 except the search direction p (fp16).  Inputs are cast to
fp16 on the host (halves HBM traffic).  The four big tensors stream through
the gpsimd SWDGE queue, which stripes descriptors over all 16 SDMA engines
(the HWDGE rings only get 5); queue FIFO order gives the x-side strict
priority.  Each partition's data is contiguous in DRAM (125 descriptors
per tensor).
"""

import numpy as np

import concourse.bacc as bacc
import concourse.mybir as mybir
from concourse.bass_utils import run_bass_kernel_spmd
from concourse.tile import TileContext

F32 = mybir.dt.float32
F16 = mybir.dt.float16
F32R = mybir.dt.float32r
K = 64          # spectral basis size
C = 128         # feature channels
V = 5000        # vertices
P = 128         # DMA partition rows
NB = 39         # full contraction chunks (V = P * NB + TAIL)
TAIL = 8        # leftover rows handled by one small matmul
N_CORES = 8
N_ITERS = 6
NEWTON_STEPS_S = 7
NEWTON_STEPS_G = 4
SQRT_LMBDA = 10.0

SHARD = False   # kept for test.py compat; only the unsharded path exists

_PROGRAM_CACHE = {}


def build_program(shard: bool):
    nc = bacc.Bacc("TRN2", num_devices=N_CORES)

    CK = C + K      # concatenated row: [fx | pxT]
    xc_d = nc.dram_tensor("xc", [V, CK], F16, kind="ExternalInput")
    yc_d = nc.dram_tensor("yc", [V, CK], F16, kind="ExternalInput")
    # mx|my|mxT|I [64, 4*64] fp32 (host-concatenated; I avoids the gpsimd
    # affine_select library swap that delayed partition_all_reduce ~13us)
    small_d = nc.dram_tensor("small", [K, 4 * K], F32, kind="ExternalInput")
    ev_d = nc.dram_tensor("ev", [1, 2 * K], F32, kind="ExternalInput")
    out_d = nc.dram_tensor("out", [K, K], F32, kind="ExternalOutput")

    xc_ap = xc_d[0:P * NB, :].rearrange("(p n) c -> p n c", p=P)
    yc_ap = yc_d[0:P * NB, :].rearrange("(p n) c -> p n c", p=P)

    with TileContext(nc) as tc:
        with (
            tc.tile_pool(name="big", bufs=1) as bp,
            tc.tile_pool(name="persist", bufs=1) as sp,
            tc.tile_pool(name="work", bufs=2) as wp,
            tc.tile_pool(name="psum", bufs=2, space="PSUM") as pp,
            tc.tile_pool(name="yacc", bufs=1, space="PSUM") as yp,
            tc.tile_pool(name="scal", bufs=2, space="PSUM") as scp,
        ):
            _sc_state = {"i": 0}

            def sc_tile(shape):
                i = _sc_state["i"]
                _sc_state["i"] += 1
                return scp.tile(shape, F32, tag="sc", name=f"sct{i}")
            _ps_state = {"i": 0}

            def ps_tile(shape):
                i = _ps_state["i"]
                _ps_state["i"] += 1
                return pp.tile(shape, F32, tag=f"ps{i % 2}", name=f"pst{i}")

            # ---------------- input DMA ------------------------------------
            # smalls ride the (otherwise idle) HWDGE queues; the four big
            # tensors stream through gpsimd SWDGE in x-first FIFO order.
            small_t = sp.tile([K, 4 * K], F32)
            ev_t = sp.tile([1, 2 * K], F32)
            xc_t = bp.tile([P, NB, CK], F16)
            yc_t = bp.tile([P, NB, CK], F16)
            xtl = sp.tile([TAIL, CK], F16)
            ytl = sp.tile([TAIL, CK], F16)
            H = NB // 2
            nc.sync.dma_start(small_t, small_d[:, :])
            nc.sync.dma_start(ev_t, ev_d[:, :])
            nc.sync.dma_start(xtl, xc_d[P * NB:V, :])
            nc.sync.dma_start(ytl, yc_d[P * NB:V, :])
            nc.gpsimd.dma_start(xc_t[:, 0:H, :], xc_ap[:, 0:H, :])
            nc.gpsimd.dma_start(xc_t[:, H:NB, :], xc_ap[:, H:NB, :])
            nc.gpsimd.dma_start(yc_t[:, 0:H, :], yc_ap[:, 0:H, :])
            nc.gpsimd.dma_start(yc_t[:, H:NB, :], yc_ap[:, H:NB, :])

            mx_s = small_t[:, 0:K]
            my_s = small_t[:, K:2 * K]
            mxT_s = small_t[:, 2 * K:3 * K]
            id64 = small_t[:, 3 * K:4 * K]

            # fp16 copies of the small matrices (scalar engine)
            m16 = sp.tile([K, 3 * K], F16)
            nc.scalar.copy(m16[:, 0:K], mx_s)
            nc.scalar.copy(m16[:, K:2 * K], my_s)
            nc.scalar.copy(m16[:, 2 * K:3 * K], mxT_s)
            mx16 = m16[:, 0:K]
            my16 = m16[:, K:2 * K]
            mxT16 = m16[:, 2 * K:3 * K]

            id16 = sp.tile([K, K], F16)
            nc.scalar.copy(id16, id64)
            ones_row = sp.tile([1, K], F32)
            nc.vector.memset(ones_row, 1.0)
            ones_row16 = sp.tile([1, K], F16)
            nc.scalar.copy(ones_row16, ones_row)
            ones_col = sp.tile([K, 1], F32)
            nc.vector.memset(ones_col, 1.0)
            ones_col16 = sp.tile([K, 1], F16)
            nc.scalar.copy(ones_col16, ones_col)

            def preduce(acc16, tag):
                """sum over partitions: fp16 [K,1] -> [1,1] PSUM via PE
                matmul (avoids the gpsimd custom-op library + ~12us load).
                Feeder values are pre-scaled by 2^-8 to fit fp16; the scale
                cancels in every alpha/beta ratio."""
                t_p = sc_tile([1, 1])
                nc.tensor.matmul(t_p, acc16, ones_col16[:, :])
                return t_p

            def bcast(s11_16, tag):
                """fp16 [1,1] SBUF -> [K,1] PSUM broadcast via PE matmul"""
                b_p = sc_tile([K, 1])
                nc.tensor.matmul(b_p, ones_row16[:, :], s11_16)
                return b_p

            # ---------------- G = My^T My (early; data lands first) --------
            g_p = ps_tile([K, K])
            nc.tensor.matmul(g_p, my16, my16)
            g16 = sp.tile([K, K], F16, tag="g16", name="g16")
            nc.vector.tensor_copy(g16, g_p)

            # resolvent scalars: ev = [ex | ey]; t = ev/max(ev); im = 1/(1+t)
            # re = sqrt(t)*im; both scaled by sqrt(LMBDA)
            evmax = sp.tile([1, 1], F32)
            nc.vector.tensor_reduce(evmax, ev_t, mybir.AxisListType.X,
                                    mybir.AluOpType.max)
            evrec = sp.tile([1, 1], F32)
            nc.vector.reciprocal(evrec, evmax)
            t_t = sp.tile([1, 2 * K], F32)
            nc.vector.tensor_scalar_mul(t_t, ev_t, evrec)
            tp1 = sp.tile([1, 2 * K], F32)
            nc.vector.tensor_scalar_add(tp1, t_t, 1.0)
            im_t = sp.tile([1, 2 * K], F32)
            nc.vector.reciprocal(im_t, tp1)
            sq_t = sp.tile([1, 2 * K], F32)
            nc.scalar.sqrt(sq_t, t_t)
            re_t = sp.tile([1, 2 * K], F32)
            nc.vector.tensor_mul(re_t, sq_t, im_t)
            nc.vector.tensor_scalar_mul(re_t, re_t, SQRT_LMBDA)
            nc.vector.tensor_scalar_mul(im_t, im_t, SQRT_LMBDA)

            # Newton-Schulz inverse (S symmetric PD), two-hop steps:
            #   B = 2I - S X  (DVE STT, fp16 out);  X' = X B  (X symmetric).
            # interleave(j) fills the PE-queue gaps with projection matmuls.
            def newton_inverse(mat_p, s16, tag, steps, interleave=None):
                # X0 = I / ||S||_F  (PE partition-reduce; no gpsimd library)
                prod = wp.tile([K, K], F32, tag="dot_dm", name=f"{tag}_sq")
                sqa = sp.tile([K, 1], F16, tag=f"{tag}_rs", name=f"{tag}_rs")
                with nc.allow_low_precision(reason="fp16 PE-reduce feeder"):
                    nc.vector.scalar_tensor_tensor(
                        prod, s16, 2.0 ** -8, s16,
                        op0=mybir.AluOpType.mult, op1=mybir.AluOpType.mult,
                        accum_out=sqa)
                tf_p = preduce(sqa, tag)       # ||S||_F^2 * 2^-8
                sf = sp.tile([1, 1], F32, tag=f"{tag}_sf", name=f"{tag}_sf")
                nc.scalar.sqrt(sf, tf_p)       # ||S||_F * 2^-4
                rec = sp.tile([1, 1], F32, tag=f"{tag}_rc", name=f"{tag}_rc")
                nc.vector.reciprocal(rec, sf)
                alr = sp.tile([1, 1], F16, tag=f"{tag}_al", name=f"{tag}_al")
                nc.vector.tensor_scalar_mul(alr, rec, 2.0 ** -4)  # 1/||S||_F
                al_bc = bcast(alr, tag)
                x_s = sp.tile([K, K], F16, tag=f"{tag}_x0", name=f"{tag}_x0")
                nc.vector.tensor_scalar_mul(x_s, id64, al_bc)
                it_i = 0
                for it in range(steps):
                    t1 = ps_tile([K, K])
                    nc.tensor.matmul(t1, s16, x_s)            # S X (S sym)
                    if interleave is not None:
                        interleave(it_i); it_i += 1
                    b16 = wp.tile([K, K], F16, tag=f"{tag}_b16",
                                  name=f"{tag}_b16")
                    nc.vector.scalar_tensor_tensor(
                        b16, id64, 2.0, t1,
                        op0=mybir.AluOpType.mult,
                        op1=mybir.AluOpType.subtract)         # 2I - S X
                    xn = ps_tile([K, K])
                    nc.tensor.matmul(xn, x_s, b16)            # X (2I - S X)
                    if interleave is not None:
                        interleave(it_i); it_i += 1
                    x_new = sp.tile([K, K], F16, tag=f"{tag}_x{it + 1}",
                                    name=f"{tag}_x{it + 1}")
                    if it == steps - 1:
                        nc.scalar.copy(x_new, xn)
                    else:
                        nc.vector.tensor_copy(x_new, xn)
                    x_s = x_new
                if interleave is not None:
                    interleave(1000)   # flush any remainder
                return x_s

            # ---- x projections interleaved into Newton-G's PE gaps --------
            with tc.tile_pool(name="pacc", bufs=1, space="PSUM") as pacc:
                pj_p = pacc.tile([C, 2 * K], F32)
                at_p = pj_p[:, 0:K]              # A^T  [C,K]
                byt_p = pj_p[:, K:2 * K]         # By^T [C,K]

                xprog = {"n": 0}

                def emit_xproj(upto):
                    while xprog["n"] < min(upto, NB + 1):
                        n = xprog["n"]
                        if n < NB:
                            nc.tensor.matmul(
                                at_p, xc_t[:, n, 0:C], xc_t[:, n, C:CK],
                                start=(n == 0), stop=False)
                        else:
                            nc.tensor.matmul(
                                at_p, xtl[:, 0:C], xtl[:, C:CK],
                                start=False, stop=True)
                        xprog["n"] += 1

                gi16 = newton_inverse(
                    g_p, g16, "gi", NEWTON_STEPS_G,
                    interleave=lambda j: emit_xproj(j * 6))
                emit_xproj(NB + 1)

                at16 = sp.tile([C, K], F16, tag="at16", name="at16")
                nc.vector.tensor_copy(at16, at_p)

                # S~ = Mx^T (A A^T) Mx
                sa_p = ps_tile([K, K])
                nc.tensor.matmul(sa_p, at16, at16)          # A A^T
                sa16 = wp.tile([K, K], F16, tag="sa16", name="sa16")
                nc.vector.tensor_copy(sa16, sa_p)
                h1_p = ps_tile([K, K])
                nc.tensor.matmul(h1_p, sa16, mx16)          # S_A Mx (sym)
                h16 = wp.tile([K, K], F16, tag="h16", name="h16")
                nc.vector.tensor_copy(h16, h1_p)
                st_p = ps_tile([K, K])
                nc.tensor.matmul(st_p, mx16, h16)           # Mx^T S_A Mx
                st16 = sp.tile([K, K], F16, tag="st16", name="st16")
                nc.scalar.copy(st16, st_p)

                # ---- y projections interleaved into Newton-S's gaps -------
                yprog = {"n": 0}

                def emit_yproj(upto):
                    while yprog["n"] < min(upto, NB + 1):
                        n = yprog["n"]
                        if n < NB:
                            nc.tensor.matmul(
                                byt_p, yc_t[:, n, 0:C], yc_t[:, n, C:CK],
                                start=(n == 0), stop=False)
                        else:
                            nc.tensor.matmul(
                                byt_p, ytl[:, 0:C], ytl[:, C:CK],
                                start=False, stop=True)
                        yprog["n"] += 1

                si16 = newton_inverse(
                    st_p, st16, "si", NEWTON_STEPS_S,
                    interleave=lambda j: emit_yproj((j - 3) * 6))
                emit_yproj(NB + 1)

                byt16 = sp.tile([C, K], F16, tag="byt16", name="byt16")
                nc.vector.tensor_copy(byt16, byt_p)

            # D1T[a,i] = re2[a] - re1[i]; D2T likewise from im (emitted late
            # so the tiny mask matmuls never stall the PE queue)
            d12t_s = sp.tile([K, 2 * K], F32)
            for idx, src in enumerate((re_t, im_t)):
                pa = ps_tile([K, K])
                nc.tensor.matmul(pa, src[0:1, K:2 * K], ones_row)  # v2[p]
                pb = ps_tile([K, K])
                nc.tensor.matmul(pb, ones_row, src[0:1, 0:K])      # v1[f]
                ta = wp.tile([K, K], F32, tag=f"dta{idx}", name=f"dta{idx}")
                nc.vector.tensor_copy(ta, pa)
                nc.vector.tensor_sub(d12t_s[:, idx * K:(idx + 1) * K], ta, pb)
            d12v = d12t_s[:, :].rearrange("p (a b) -> p a b", a=2)

            # ---- RHS' = G By A^T Mx (3 matmuls, G-symmetry trick) ---------
            byat_p = ps_tile([K, K])
            nc.tensor.matmul(byat_p, byt16, at16)       # By A^T
            byat16 = wp.tile([K, K], F16, tag="byat16", name="byat16")
            nc.scalar.copy(byat16, byat_p)
            s2_p = ps_tile([K, K])
            nc.tensor.matmul(s2_p, byat16, g16)         # (G ByA^T)^T (G sym)
            s2c = wp.tile([K, K], F16, tag="s2c", name="s2c")
            nc.scalar.copy(s2c, s2_p)
            rp_p = ps_tile([K, K])
            nc.tensor.matmul(rp_p, s2c, mx16)           # G ByA^T Mx

            # ------- PCG state ---------------------------------------------
            # rz_s = [r | z] fp32 (fused alpha-update); p16 fp16 direction;
            # q_s, s_s fp32; y accumulates Y^T in PSUM via matmul.
            rz_s = sp.tile([K, 2 * K], F32)
            w_s = sp.tile([K, K], F32)
            u16 = sp.tile([K, 2 * K], F16)
            p16 = sp.tile([K, K], F16)
            y_p = yp.tile([K, K], F32)
            r_sl = rz_s[:, 0:K]
            z_sl = rz_s[:, K:2 * K]
            qs_v = sp.tile([K, 2 * K], F32)   # [q | s] fused tile
            q_s = qs_v[:, 0:K]
            s_s = qs_v[:, K:2 * K]
            u16v = u16[:, :].rearrange("p (a b) -> p a b", a=2)
            z_bc = z_sl.rearrange("p (o b) -> p o b", o=1).broadcast_to(
                [K, 2, K])

            nc.vector.tensor_copy(r_sl, rp_p)
            r16 = wp.tile([K, K], F16, tag="x16", name="r16i")
            nc.scalar.copy(r16, rp_p)

            def precond_psum(x16, tag):
                """P^-1 x in PSUM via (Gi x)^T = mm(lhsT=x16, rhs=Gi)."""
                ut_p = ps_tile([K, K])
                nc.tensor.matmul(ut_p, x16, gi16)
                ut16 = wp.tile([K, K], F16, tag=f"{tag}_ut", name=f"{tag}_ut")
                nc.scalar.copy(ut16, ut_p)
                v_p = ps_tile([K, K])
                nc.tensor.matmul(v_p, ut16, si16)
                return v_p

            def matvec_z(z16, tag):
                """w = M z -> w_s; w16 returned for the precond."""
                nc.vector.tensor_mul(u16v, d12v, z_bc)   # [D1T*z | D2T*z]
                gzt_p = ps_tile([K, K])
                nc.tensor.matmul(gzt_p, z16, g16)         # (G z)^T
                gzt16 = wp.tile([K, K], F16, tag="mv_gzt", name="mv_gzt")
                nc.vector.tensor_copy(gzt16, gzt_p)
                gu_p = ps_tile([K, 3 * K])
                nc.tensor.matmul(gu_p[:, 0:2 * K], g16, u16)   # G u (both)
                nc.tensor.matmul(gu_p[:, 2 * K:3 * K], gzt16, st16)  # (Gz)S~
                mm_s = wp.tile([K, 2 * K], F32, tag="mv_mm", name="mv_mm")
                nc.vector.tensor_mul(mm_s, d12t_s, gu_p[:, 0:2 * K])  # mask
                a1_s = wp.tile([K, K], F32, tag="mv_a1", name="mv_a1")
                nc.vector.tensor_add(a1_s, mm_s[:, 0:K], mm_s[:, K:2 * K])
                nc.vector.tensor_add(w_s, a1_s, gu_p[:, 2 * K:3 * K])
                w16 = wp.tile([K, K], F16, tag="x16", name=f"{tag}_w16")
                nc.scalar.copy(w16, w_s)
                return w16

            def dot_b(a_ap, b_ap, tag):
                """<a,b> as [1,1] PSUM (DVE accumulate + PE reduce)."""
                prod = wp.tile([K, K], F32, tag="dot_dm", name="dot_dm")
                acc = wp.tile([K, 1], F16, tag=f"{tag}_acc",
                              name=f"{tag}_acc")
                with nc.allow_low_precision(reason="fp16 PE-reduce feeder"):
                    nc.vector.scalar_tensor_tensor(
                        prod, a_ap, 2.0 ** -8, b_ap,
                        op0=mybir.AluOpType.mult, op1=mybir.AluOpType.mult,
                        accum_out=acc)
                return preduce(acc, tag)

            # init: z = P^-1 r; p = z; w = Mz; v = P^-1 w; q = w, s = v
            z0_p = precond_psum(r16, "pcz")
            nc.vector.tensor_copy(z_sl, z0_p)
            nc.scalar.copy(p16, z0_p)
            z16 = wp.tile([K, K], F16, tag="z16", name="z16i")
            nc.scalar.copy(z16, z0_p)
            rz_p = dot_b(r_sl, z_sl, "rz")
            rz11 = wp.tile([1, 1], F32, tag="rz11", name="rz11")
            nc.vector.tensor_copy(rz11, rz_p)
            rzrec = wp.tile([1, 1], F32, tag="rzrec", name="rzrec")
            nc.vector.reciprocal(rzrec, rz_p)
            rzneg = wp.tile([1, 1], F32, tag="rzneg", name="rzneg")
            nc.vector.tensor_scalar_mul(rzneg, rz_p, -1.0)
            w16 = matvec_z(z16, "init")
            nc.vector.tensor_copy(q_s, w_s)
            v_p = precond_psum(w16, "pcv")
            nc.vector.tensor_copy(s_s, v_p)

            for it in range(N_ITERS):
                # ---- alpha = rz/<p,q>; fused [r|z] -= alpha [q|s] ----
                pq_p = dot_b(p16, q_s, "pq")
                pqr = wp.tile([1, 1], F32, tag="pqr", name="pqr")
                nc.vector.reciprocal(pqr, pq_p)
                def emit_y_update():
                    # y^T += alpha p^T (PSUM accumulate, off critical path;
                    # must be emitted BEFORE the p16 beta-update)
                    al = wp.tile([1, 1], F16, tag="al", name="al")
                    nc.vector.tensor_mul(al, rz11, pqr)
                    al_bc = bcast(al, "al")
                    ida = wp.tile([K, K], F16, tag="ida", name="ida")
                    nc.vector.tensor_scalar_mul(ida, id16, al_bc)
                    nc.tensor.matmul(y_p, p16, ida,
                                     start=(it == 0),
                                     stop=(it == N_ITERS - 1))

                if it < N_ITERS - 1:
                    an = wp.tile([1, 1], F16, tag="an", name="an")
                    nc.vector.tensor_mul(an, rzneg, pqr)
                    an_bc = bcast(an, "an")
                    nc.vector.scalar_tensor_tensor(
                        rz_s, qs_v, an_bc, rz_s,
                        op0=mybir.AluOpType.mult, op1=mybir.AluOpType.add)
                    z16 = wp.tile([K, K], F16, tag="z16", name=f"z16_{it}")
                    nc.vector.tensor_copy(z16, z_sl)

                    # ---- rz_new; matvec + precond for next q,s ----
                    rznew_p = dot_b(r_sl, z_sl, "rz")
                    bt = wp.tile([1, 1], F16, tag="bt", name="bt")
                    nc.vector.tensor_mul(bt, rznew_p, rzrec)
                    bt_bc = bcast(bt, "bt")
                    w16 = matvec_z(z16, f"i{it}")
                    emit_y_update()   # uses this iteration's rz11 (alpha)
                    rz11 = wp.tile([1, 1], F32, tag="rz11", name="rz11")
                    nc.vector.tensor_copy(rz11, rznew_p)
                    rzrec = wp.tile([1, 1], F32, tag="rzrec", name="rzrec")
                    nc.vector.reciprocal(rzrec, rznew_p)
                    rzneg = wp.tile([1, 1], F32, tag="rzneg", name="rzneg")
                    nc.vector.tensor_scalar_mul(rzneg, rznew_p, -1.0)
                    if it < N_ITERS - 2:
                        v_s = wp.tile([K, K], F32, tag="v_s", name="v_s")
                        v_p = precond_psum(w16, "pcv")
                        nc.scalar.copy(v_s, v_p)
                    nc.vector.scalar_tensor_tensor(
                        p16, p16, bt_bc, z_sl,
                        op0=mybir.AluOpType.mult, op1=mybir.AluOpType.add)
                    nc.vector.scalar_tensor_tensor(
                        q_s, q_s, bt_bc, w_s,
                        op0=mybir.AluOpType.mult, op1=mybir.AluOpType.add)
                    if it < N_ITERS - 2:
                        nc.vector.scalar_tensor_tensor(
                            s_s, s_s, bt_bc, v_s,
                            op0=mybir.AluOpType.mult, op1=mybir.AluOpType.add)
                else:
                    emit_y_update()
                    break

            # -------- output: C = Y Mx^T  (y_p holds Y^T) ------------------
            y16 = wp.tile([K, K], F16, tag="y16", name="y16")
            nc.scalar.copy(y16, y_p)
            c_p = ps_tile([K, K])
            nc.tensor.matmul(c_p, y16, mxT16)           # (Y^T)^T Mx^T
            c_s = wp.tile([K, K], F32, tag="c_s", name="c_s")
            nc.vector.tensor_copy(c_s, c_p)
            nc.sync.dma_start(out_d[:, :], c_s)

    nc.finalize()
    return nc


def get_program(shard: bool = False):
    if shard not in _PROGRAM_CACHE:
        _PROGRAM_CACHE[shard] = build_program(shard)
    return _PROGRAM_CACHE[shard]


def make_in_maps(inputs, shard: bool = False):
    fx = np.asarray(inputs["feat_x"], np.float32)[0]
    fy = np.asarray(inputs["feat_y"], np.float32)[0]
    pxT = np.asarray(inputs["evecs_trans_x"], np.float32)[0].T
    pyT = np.asarray(inputs["evecs_trans_y"], np.float32)[0].T
    xc = np.ascontiguousarray(
        np.concatenate([fx, pxT], axis=1)).astype(np.float16)
    yc = np.ascontiguousarray(
        np.concatenate([fy, pyT], axis=1)).astype(np.float16)
    mx = np.asarray(inputs["sqrtMk_x"], np.float32)[0]
    my = np.asarray(inputs["sqrtMk_y"], np.float32)[0]
    small = np.ascontiguousarray(np.concatenate(
        [mx, my, mx.T, np.eye(64, dtype=np.float32)], axis=1))
    ev = np.ascontiguousarray(np.concatenate([
        np.asarray(inputs["evals_x"], np.float32)[0],
        np.asarray(inputs["evals_y"], np.float32)[0],
    ])[None, :])
    m = {"xc": xc, "yc": yc, "small": small, "ev": ev}
    return [dict(m) for _ in range(N_CORES)]


def kernel(**inputs) -> np.ndarray:
    nc = get_program(SHARD)
    in_maps = make_in_maps(inputs, SHARD)
    res = run_bass_kernel_spmd(nc, in_maps, core_ids=list(range(N_CORES)))
    out = np.asarray(res.results[0]["out"], dtype=np.float32)
    return out[None]
